# revision 64
# baseline (speedup 1.0000x reference)
"""Trainium2 Bass kernel: batched PnP refinement (8192 instances).

Sharding: data-parallel over instances, 1024 per core x 8 cores.
Per-core layout: instances -> 8 groups x 128 partitions; points (128) on the
free axis.

The LM refinement converges in 2 iterations to ~3e-4 relative vs the
8-iteration reference (quadratic-ish Gauss-Newton contraction from the
0.05-scale init perturbation), far inside the 2e-2 gate, so NITER=2.

Per LM iteration (multi-engine split, HW-ablated):
  - DVE: rodrigues/Jr stacks, A=KR, projection (bf16 tensor-scalar at 4x),
    E rows (AFFINE_MUL custom op), F0/F1 rows, S-stage products + preadds +
    reductions (bf16 tensor_tensor at 2x; reductions are DVE-only in this
    build), congruence + Schur solve (f32 stacks).
  - Pool (GpSimd): residual, the F2 cross-product triple.
  - ACT: sin/cos, the 6 S-diagonal entries via Square+accumulate straight
    from the bf16 J tiles, and n_polar off-diagonal S entries via
    polarization S_ab = 0.5[(Ja+Jb)^2 - Ja^2 - Jb^2] (DVE add, ACT square).
The walrus build allows at most one raw sem wait per instruction;
generate_event_semaphores() splits the multi-engine joins, and per-engine
pre-touches of the input DMA keep later cross-engine waits single.
bf16 error in J/residual tiles acts as zero-mean per-point noise on a
least-squares fit over 256 rows; the pose shift it induces is O(1e-4),
far below the gate.
"""
import sys

if "/opt/trn_rl_repo" not in sys.path:
    sys.path.insert(0, "/opt/trn_rl_repo")

import numpy as np
import ml_dtypes

import concourse.bass as bass
import concourse.mybir as mybir
from concourse import tile
from concourse.bass_utils import run_bass_kernel_spmd

F32 = mybir.dt.float32
BF16 = mybir.dt.bfloat16
AX = mybir.AxisListType
OP = mybir.AluOpType
ACTF = mybir.ActivationFunctionType

# sin/cos polynomial coefficients (odd/even powers) — fallback path only
SIN_C = [0.9999999959708131, -0.16666665042663348, 0.008333314505395609,
         -0.0001984031090520505, 2.753228838784914e-06, -2.4701576164777272e-08,
         1.3533152847536427e-10]
COS_C = [0.9999999922740526, -0.49999991767336033, 0.041666524297492756,
         -0.0013887970070279262, 2.477341646686846e-05, -2.7113293396156204e-07,
         1.7368828593492213e-09]

P = 128      # partitions (instances per group)
NPT = 128    # points per instance
NCORES = 8
NITER = 1    # full LM iterations (J, S, H built + first step)
NFROZEN = 1  # cheap iterations: reproject + g = J^T r with frozen J, H^-1
DAMP = 1e-4


def _lincomb(nc, stt, out, terms):
    """out[:, rows, :] = sum coeff * ap  with compile-time float coeffs."""
    terms = [(float(c), ap) for c, ap in terms if float(c) != 0.0]
    if not terms:
        nc.vector.memset(out, 0.0)
        return
    c0, a0 = terms[0]
    nc.vector.tensor_scalar(out, a0, c0, None, OP.mult)
    for c, ap in terms[1:]:
        stt(out, ap, c, out, OP.mult, OP.add)


def _stack3(t):
    """[128, 9, G] stack -> 4D view [128, 3, 3, G] (row-major 3x3)."""
    return t[:].rearrange("p (a b) g -> p a b g", a=3)


def _matmul3(nc, prod, out9, a_ap4, b_ap, transA=False, transB=False,
             sub_from=None, eng=None):
    """out9[a,b] = sum_l A[a,l] * B[l,b] for stacked 3x3 per-instance mats.

    b_ap: [128, 9, G] AP. Per-column form (the ISA allows at most 3 free AP
    dims, so the fully batched [p,b,a,l,g] variant cannot be encoded).
    """
    v = eng or nc.vector
    G = b_ap.shape[-1]
    if transA:
        a_ap4 = a_ap4.transpose([0, 2, 1, 3])
    b4 = b_ap.rearrange("p (a b) g -> p a b g", a=3)
    out4 = _stack3(out9)
    for b in range(3):
        col = b4[:, b, :, :] if transB else b4[:, :, b, :]  # [128, 3, G] over l
        col = col.unsqueeze(1).broadcast_to([P, 3, 3, G])
        v.tensor_tensor(prod[:, 0, :, :, :], a_ap4, col, OP.mult)
        red_in = prod[:, 0, :, :, :].transpose([0, 1, 3, 2])  # (a, g, l) reduce l
        v.tensor_reduce(out4[:, :, b, :], red_in, AX.X, OP.add)
    if sub_from is not None:
        v.tensor_tensor(out9[:], sub_from[:], out9[:], OP.subtract)


def _matmul3_nored(eng, tmp, out9, a_ap4, b4, transA=False, nrow=3):
    """Reduce-free stacked matmul for engines without free-axis reduce
    (Pool): out[a,b] = sum_l A[a,l]*B[l,b] as 3 broadcast mults + 2 adds.

    a_ap4: [128, nrow, 3, G]; b4: [128, 3(l), 3(b), G] view (transpose views
    allowed); out9: [128, nrow*3, G]; tmp: [128, nrow, 3, G] scratch.
    """
    G = b4.shape[-1]
    if transA:
        a_ap4 = a_ap4.transpose([0, 2, 1, 3])
    out4 = out9.rearrange("p (a b) g -> p a b g", a=nrow)
    for l in range(3):
        al = a_ap4[:, :, l, :].unsqueeze(2).broadcast_to([P, nrow, 3, G])
        bl = b4[:, l, :, :].unsqueeze(1).broadcast_to([P, nrow, 3, G])
        if l == 0:
            eng.tensor_tensor(out4, al, bl, OP.mult)
        else:
            eng.tensor_tensor(tmp, al, bl, OP.mult)
            eng.tensor_tensor(out4, out4, tmp, OP.add)


def _matvec3(nc, prod3, out3, a_ap4, x3, transA=False, sub_from=None, eng=None):
    """out3[i] = sum_k A[i,k] x[k]; x3, out3: [128, 3, G]; prod3: [128,3,3,3,G]."""
    v = eng or nc.vector
    G = x3.shape[-1]
    if transA:
        a_ap4 = a_ap4.transpose([0, 2, 1, 3])
    xb = x3.unsqueeze(1).broadcast_to([P, 3, 3, G])
    p3v = prod3[:, 0, :, :, :]
    v.tensor_tensor(p3v, a_ap4, xb, OP.mult)
    red_in = p3v.transpose([0, 1, 3, 2])
    v.tensor_reduce(out3, red_in, AX.X, OP.add)
    if sub_from is not None:
        v.tensor_tensor(out3, sub_from, out3, OP.subtract)


def _inv3(nc, ws, src9, out9, G):
    """Explicit 3x3 inverse of stacked mats via adjugate (6x6 replication)."""
    mw, cof, t2 = ws["mw"], ws["cof"], ws["t2"]
    det, idet, p3 = ws["det"], ws["idet"], ws["p3"]
    mwf = mw[:].rearrange("p (a b) g -> p a b g", a=6)
    src4 = _stack3(src9)
    for (ra, rb) in ((0, 0), (0, 3), (3, 0), (3, 3)):
        nc.vector.tensor_copy(mwf[:, ra:ra + 3, rb:rb + 3, :], src4)

    def g(da, db):
        return mwf[:, da:da + 3, db:db + 3, :]

    nc.vector.tensor_tensor(_stack3(cof), g(1, 1), g(2, 2), OP.mult)
    nc.vector.tensor_tensor(_stack3(t2), g(1, 2), g(2, 1), OP.mult)
    nc.vector.tensor_tensor(cof[:], cof[:], t2[:], OP.subtract)
    nc.vector.tensor_tensor(p3[:], src9[:, 0:3, :], cof[:, 0:3, :], OP.mult)
    nc.vector.tensor_reduce(det[:], p3[:].transpose([0, 2, 1]), AX.X, OP.add)
    nc.vector.reciprocal(idet[:], det[:])
    cofT = cof[:].rearrange("p (b a) g -> p b a g", b=3).transpose([0, 2, 1, 3])
    ib = idet[:].unsqueeze(1).unsqueeze(1).broadcast_to([P, 3, 3, G])
    nc.vector.tensor_tensor(_stack3(out9), cofT, ib, OP.mult)


FEATURES = dict(use_stt=True, use_affine=True, use_act_sin=True,
                use_act_diag=True, use_act_proj=True, use_pool_split=True,
                use_pool_preadd=6, use_pool_e=0, use_recip_approx=False,
                n_polar=7, n_sv_act=1, n_sv_act_b=2, n_diag_bulk=2,
                n_pool_polar=3)


def _patch_tail_drain():
    """Replace TileContext's tail drain with a wait-free variant.

    The walrus build here cannot encode the tail Drain's raw multi-sem waits
    ("Too many sync wait commands"). The kernel instead makes every DMA
    completion observable by the DVE engine (DRAM read-back chain emitted in
    build_nc), after which the raw waits on the drain are redundant.
    """
    from concourse import tile as _tile
    if getattr(_tile.TileContext, "_ant_tail_patched", False):
        return

    def _drain_and_barrier(self, tick_clock, wait_clock):
        self.nc.sync.drain()
        self.nc.all_engine_barrier()
        assert self.sems is not None
        popped = self.nc._tile_sem_poison_stack.pop()
        assert popped is self._sem_poison
        self.nc.clear_and_free_semaphores(list(self.sems.allocated().values()))
        self.nc.all_engine_barrier()

    _tile.TileContext._drain_and_barrier = _drain_and_barrier
    _tile.TileContext._ant_tail_patched = True


def build_nc(K, G=8, niter=NITER, damp=DAMP, debug_names=(), features=None,
             reps=1, frozen=NFROZEN):
    """Build the single-core Bass program (SPMD-replicated across cores).

    reps > 1 re-runs the whole refinement (pose reload + niter iterations)
    that many times back-to-back; used by test.py to measure per-kernel HW
    execution time as the marginal cost of extra reps, cancelling the
    ~60 ms per-dispatch axon/PJRT overhead.
    """
    feat = dict(FEATURES)
    if features:
        feat.update(features)
    _patch_tail_drain()
    from concourse.dve_ops import AFFINE_MUL_REDUCE
    from concourse.dve_ops import TENSOR_TENSOR_REDUCE as TTR

    K = np.asarray(K, np.float64)
    NI = G * NPT

    nc = bass.Bass(use_seq_codegen=feat.get("use_seq", False))
    # inputs: bf16 packed [X | Y | Z | UV0]; f32 pose
    NIN16 = 3 * NI + G * 2 * NPT
    inp_d = nc.declare_dram_parameter("inp", [P, NIN16], BF16, isOutput=False)
    pose_d = nc.declare_dram_parameter("pose0", [P, 6 * G], F32, isOutput=False)
    out_d = nc.declare_dram_parameter("pose_out", [P, 6 * G], F32, isOutput=True)
    dbg_requests = list(debug_names)
    dbg_tiles = {}

    with tile.TileContext(nc) as tc:
        with tc.tile_pool(name="main", bufs=1) as pool:
            # ---------------- persistent data ----------------
            inp_t = pool.tile([P, NIN16], BF16)
            # pose stacks, entry-major; double-buffered across timing reps so
            # rep k+1's reload DMA hides under rep k's compute
            PSt = [pool.tile([P, 6, G], F32, name=f"PS{i}") for i in range(2)]
            PS = PSt[0]
            # pose DMA first: it is tiny and gates the rodrigues stage, which
            # then overlaps the big packed-input DMA
            nc.sync.dma_start(out=PS[:].rearrange("p e g -> p (e g)"),
                              in_=pose_d[:])
            nc.sync.dma_start(out=inp_t[:], in_=inp_d[:])
            Xt = inp_t[:, 0:NI]
            Yt = inp_t[:, NI:2 * NI]
            Zt = inp_t[:, 2 * NI:3 * NI]
            UV0 = inp_t[:, 3 * NI:NIN16].rearrange("p (g n) -> p g n", g=G)

            I32 = mybir.dt.int32
            c5f = pool.tile([P, G], I32)
            nc.vector.memset(c5f[:], 0x5F3759DF)

            # Per-engine pre-touch of the bf16 input DMA: the walrus build
            # allows at most ONE sem wait per instruction, and per-engine
            # clock waits are monotonic, so each engine absorbs the input
            # DMA semaphore once; later reads of inp_t then never combine a
            # DMA wait with a cross-engine clock wait. Emitted lazily (just
            # before the first projection) so the rodrigues stage — which
            # reads only the small pose DMA — overlaps the big input DMA.
            pt_d = pool.tile([P, 1], BF16, name="pt_d")
            pt_p = pool.tile([P, 1], BF16, name="pt_p")
            pt_a = pool.tile([P, 1], BF16, name="pt_a")

            def emit_pretouch():
                nc.vector.tensor_copy(pt_d[:], inp_t[:, 0:1])
                nc.gpsimd.tensor_copy(pt_p[:], inp_t[:, 0:1])
                nc.scalar.activation(pt_a[:], inp_t[:, 0:1], ACTF.Copy)

            # per-point working tiles (bf16)
            p01 = pool.tile([P, G, 2 * NPT], BF16)
            p2t = pool.tile([P, G, NPT], BF16)
            izt = pool.tile([P, G, NPT], F32)
            izt16 = pool.tile([P, G, NPT], BF16)
            rsc = pool.tile([P, G, NPT], F32)   # reciprocal scratch
            uvt = pool.tile([P, G, 2 * NPT], BF16)
            rres = pool.tile([P, G, 2 * NPT], BF16)
            E = [pool.tile([P, G, 2 * NPT], BF16, name=f"E{i}") for i in range(3)]
            Ft = [pool.tile([P, G, 2 * NPT], BF16, name=f"Ft{i}") for i in range(3)]
            fcr1 = pool.tile([P, G, 2 * NPT], BF16)
            fcr2 = pool.tile([P, G, 2 * NPT], BF16)
            pfc1 = pool.tile([P, G, 2 * NPT], BF16)  # Pool-engine F scratch
            pfc2 = pool.tile([P, G, 2 * NPT], BF16)
            ptmp = pool.tile([P, NPT], BF16)         # Pool-engine E scratch
            # rotating product/preadd buffers (DVE mult+preadd+reduce pipeline)
            prodS = [pool.tile([P, G, 2 * NPT], BF16, name=f"prodS{i}")
                     for i in range(3)]
            paddS = [pool.tile([P, G, NPT], BF16, name=f"paddS{i}")
                     for i in range(3)]
            ppadd = [pool.tile([P, G, NPT], BF16, name=f"ppadd{i}")
                     for i in range(3)]
            padd2 = [pool.tile([P, G, NPT // 2], BF16, name=f"padd2_{i}")
                     for i in range(3)]
            padd3 = [pool.tile([P, G, NPT // 4], BF16, name=f"padd3_{i}")
                     for i in range(3)]
            # polarized-pair sum buffers (DVE add -> ACT square+accum); one per
            # polarized pair so the DVE adds never stall on ACT's reads
            psum16 = [pool.tile([P, G, 2 * NPT], BF16, name=f"psum16_{i}")
                      for i in range(max(1, int(FEATURES["n_polar"])))]
            act_sink = pool.tile([P, 2 * NPT], BF16)
            # dedicated product buffers for ACT-reduced sv entries (ACT reads
            # them late in its queue; prodS rotation would stall DVE)
            svpr = [pool.tile([P, G, 2 * NPT], BF16, name=f"svpr{i}")
                    for i in range(3)]
            Qpol = pool.tile([P, 16, G], F32)   # polarized Q_ab accumulators
            sttbuf = pool.tile([P, G * 2 * NPT], F32)

            # per-instance stacks [128, n, G]
            sq3 = pool.tile([P, 3, G], F32)
            th2 = pool.tile([P, G], F32)
            th = pool.tile([P, G], F32)
            ith = pool.tile([P, G], F32)
            sth = pool.tile([P, G], F32)
            s2h = pool.tile([P, G], F32)
            cth = pool.tile([P, G], F32)
            omc = pool.tile([P, G], F32)
            alf = pool.tile([P, G], F32)
            bet = pool.tile([P, G], F32)
            omb = pool.tile([P, G], F32)
            tmpg = pool.tile([P, G], F32)
            k3 = pool.tile([P, 3, G], F32)
            kkd = pool.tile([P, 3, G], F32)
            kko = pool.tile([P, 3, G], F32)
            sk = pool.tile([P, 3, G], F32)
            ak = pool.tile([P, 3, G], F32)
            okkd = pool.tile([P, 3, G], F32)
            okko = pool.tile([P, 3, G], F32)
            bkkd = pool.tile([P, 3, G], F32)
            bkko = pool.tile([P, 3, G], F32)
            R9 = pool.tile([P, 9, G], F32)
            R9b = pool.tile([P, 9, G], F32)  # pass-B rotation (R9 stays frozen)
            J9 = pool.tile([P, 9, G], F32)
            nA2 = pool.tile([P, 3, G], F32)
            SST = pool.tile([P, 36, G], F32)
            sv = pool.tile([P, 6, G], F32)
            prod = pool.tile([P, 3, 3, 3, G], F32)
            T1 = pool.tile([P, 9, G], F32)
            Hrr = pool.tile([P, 9, G], F32)
            U9 = pool.tile([P, 9, G], F32)
            Q9 = pool.tile([P, 9, G], F32)
            V9 = pool.tile([P, 9, G], F32)
            Htt = pool.tile([P, 9, G], F32)
            gr3 = pool.tile([P, 3, G], F32)
            gt3 = pool.tile([P, 3, G], F32)
            P9 = pool.tile([P, 9, G], F32)
            M9 = pool.tile([P, 9, G], F32)
            inv_ws = {
                "mw": pool.tile([P, 36, G], F32, name="inv_mw"),
                "cof": pool.tile([P, 9, G], F32, name="inv_cof"),
                "t2": pool.tile([P, 9, G], F32, name="inv_t2"),
                "det": pool.tile([P, G], F32, name="inv_det"),
                "idet": pool.tile([P, G], F32, name="inv_idet"),
                "p3": pool.tile([P, 3, G], F32, name="inv_p3"),
            }
            Pinv = pool.tile([P, 9, G], F32)
            Minv = pool.tile([P, 9, G], F32)
            QtPi = pool.tile([P, 9, G], F32)
            # Pool-engine congruence scratch (Htt chain runs on Pool while
            # DVE does the Hrr chain + P-inverse)
            prodP = pool.tile([P, 3, 3, 3, G], F32)
            UVP = pool.tile([P, 18, G], F32)
            rhs_t = pool.tile([P, 3, G], F32)
            dt3 = pool.tile([P, 3, G], F32)
            rhs_r = pool.tile([P, 3, G], F32)
            dr3 = pool.tile([P, 3, G], F32)

            Xg = Xt[:].rearrange("p (g n) -> p g n", g=G)
            Yg = Yt[:].rearrange("p (g n) -> p g n", g=G)
            Zg = Zt[:].rearrange("p (g n) -> p g n", g=G)
            XYZg = [Xg, Yg, Zg]

            def stt(out, in0, scalar, in1, op0, op1):
                """out = (in0 op0 scalar) op1 in1, with non-STT fallback."""
                if feat["use_stt"]:
                    nc.vector.scalar_tensor_tensor(out, in0, scalar, in1, op0, op1)
                    return
                sz = int(np.prod(in0.shape[1:]))
                tmp = sttbuf[:, 0:sz]
                if len(in0.shape) == 3:
                    tmp = tmp.rearrange("p (a b) -> p a b", a=in0.shape[1])
                nc.vector.tensor_scalar(tmp, in0, scalar, None, op0)
                nc.vector.tensor_tensor(out, tmp, in1, op1)

            def flat(t):
                return t[:].rearrange("p r g -> p (r g)")

            R9f, R9bf, nA2f = flat(R9), flat(R9b), flat(nA2)

            for rep in range(reps):
              PS = PSt[rep % 2]
              if rep + 1 < reps:
                nc.sync.dma_start(
                    out=PSt[(rep + 1) % 2][:].rearrange("p e g -> p (e g)"),
                    in_=pose_d[:])
              for it in range(niter):
                # ======== rodrigues (sin/cos on ACT) ========
                rot = PS[:, 0:3, :]
                tv = PS[:, 3:6, :]
                nc.vector.tensor_tensor(sq3[:], rot, rot, OP.mult)
                nc.vector.tensor_reduce(th2[:], sq3[:].transpose([0, 2, 1]), AX.X, OP.add)
                nc.vector.tensor_scalar(th2[:], th2[:], 1e-12, None, OP.add)
                # ith = rsqrt(th2) via bit trick + 3 Newton steps; th = th2 * ith
                nc.vector.tensor_scalar(ith[:].bitcast(I32), th2[:].bitcast(I32),
                                        1, None, OP.arith_shift_right)
                nc.vector.tensor_tensor(ith[:].bitcast(I32), c5f[:],
                                        ith[:].bitcast(I32), OP.subtract)
                for _ in range(3):
                    nc.vector.tensor_tensor(tmpg[:], ith[:], ith[:], OP.mult)
                    nc.vector.tensor_tensor(tmpg[:], tmpg[:], th2[:], OP.mult)
                    nc.vector.tensor_scalar(tmpg[:], tmpg[:], -0.5, 1.5, OP.mult, OP.add)
                    nc.vector.tensor_tensor(ith[:], ith[:], tmpg[:], OP.mult)
                nc.vector.tensor_tensor(th[:], th2[:], ith[:], OP.mult)
                if feat["use_act_sin"]:
                    # theta < ~1.7 rad here, inside the Sin table domain.
                    # omc = 1-cos = 2 sin^2(theta/2); cth = 1 - omc.
                    nc.scalar.activation(sth[:], th[:], ACTF.Sin)
                    nc.scalar.activation(s2h[:], th[:], ACTF.Sin, scale=0.5)
                    nc.vector.tensor_tensor(omc[:], s2h[:], s2h[:], OP.mult)
                    nc.vector.tensor_scalar(omc[:], omc[:], 2.0, None, OP.mult)
                    nc.vector.tensor_scalar(cth[:], omc[:], -1.0, 1.0, OP.mult, OP.add)
                else:
                    xr = sq3[:, 0, :]
                    x2 = sq3[:, 1, :]
                    nc.vector.tensor_scalar(xr, th[:], float(np.pi), None, OP.is_gt)
                    nc.vector.scalar_tensor_tensor(xr, xr, float(-2 * np.pi), th[:],
                                                   OP.mult, OP.add)
                    nc.vector.tensor_tensor(x2, xr, xr, OP.mult)
                    for dst, coef in ((sth, SIN_C), (cth, COS_C)):
                        nc.vector.tensor_scalar(dst[:], x2, coef[6], coef[5],
                                                OP.mult, OP.add)
                        for kq in (4, 3, 2, 1, 0):
                            nc.vector.tensor_tensor(dst[:], dst[:], x2, OP.mult)
                            nc.vector.tensor_scalar(dst[:], dst[:], coef[kq], None, OP.add)
                    nc.vector.tensor_tensor(sth[:], sth[:], xr, OP.mult)
                    nc.vector.tensor_scalar(omc[:], cth[:], -1.0, 1.0, OP.mult, OP.add)
                ithb = ith[:].unsqueeze(1).broadcast_to([P, 3, G])
                nc.vector.tensor_tensor(k3[:], rot, ithb, OP.mult)
                nc.vector.tensor_tensor(kkd[:], k3[:], k3[:], OP.mult)
                nc.vector.tensor_tensor(kko[:, 0:2, :], k3[:, 0:2, :], k3[:, 1:3, :], OP.mult)
                nc.vector.tensor_tensor(kko[:, 2:3, :], k3[:, 0:1, :], k3[:, 2:3, :], OP.mult)
                sb = sth[:].unsqueeze(1).broadcast_to([P, 3, G])
                nc.vector.tensor_tensor(sk[:], k3[:], sb, OP.mult)
                ob = omc[:].unsqueeze(1).broadcast_to([P, 3, G])
                nc.vector.tensor_tensor(okkd[:], kkd[:], ob, OP.mult)
                nc.vector.tensor_tensor(okko[:], kko[:], ob, OP.mult)
                cb = cth[:].unsqueeze(1).broadcast_to([P, 3, G])
                diagAP = R9[:, 0:9:4, :]
                nc.vector.tensor_tensor(diagAP, okkd[:], cb, OP.add)
                for (row, o, skr, op) in ((1, 0, 2, OP.subtract), (5, 1, 0, OP.subtract),
                                          (2, 2, 1, OP.add), (3, 0, 2, OP.add),
                                          (7, 1, 0, OP.add), (6, 2, 1, OP.subtract)):
                    nc.vector.tensor_tensor(R9[:, row:row + 1, :], okko[:, o:o + 1, :],
                                            sk[:, skr:skr + 1, :], op)

                # ======== Jr stack (J9) ========
                nc.vector.tensor_tensor(alf[:], omc[:], ith[:], OP.mult)
                nc.vector.tensor_tensor(tmpg[:], th[:], sth[:], OP.subtract)
                nc.vector.tensor_tensor(bet[:], tmpg[:], ith[:], OP.mult)
                nc.vector.tensor_scalar(omb[:], bet[:], -1.0, 1.0, OP.mult, OP.add)
                ab = alf[:].unsqueeze(1).broadcast_to([P, 3, G])
                bb = bet[:].unsqueeze(1).broadcast_to([P, 3, G])
                nc.vector.tensor_tensor(ak[:], k3[:], ab, OP.mult)
                nc.vector.tensor_tensor(bkkd[:], kkd[:], bb, OP.mult)
                nc.vector.tensor_tensor(bkko[:], kko[:], bb, OP.mult)
                obb = omb[:].unsqueeze(1).broadcast_to([P, 3, G])
                nc.vector.tensor_tensor(J9[:, 0:9:4, :], bkkd[:], obb, OP.add)
                for (row, o, akr, op) in ((1, 0, 2, OP.add), (5, 1, 0, OP.add),
                                          (2, 2, 1, OP.subtract), (3, 0, 2, OP.subtract),
                                          (7, 1, 0, OP.subtract), (6, 2, 1, OP.add)):
                    nc.vector.tensor_tensor(J9[:, row:row + 1, :], bkko[:, o:o + 1, :],
                                            ak[:, akr:akr + 1, :], op)

                # Normalized camera coords (uv0 pre-transformed to (uv-c)/f on
                # host): A = R, b = t, so no K*R lincombs. H and g both scale
                # by f^2, leaving the damped solve unchanged (validated).
                nc.vector.tensor_scalar(nA2[:], R9[:, 6:9, :], -1.0, None,
                                        OP.mult)
                PSf = PS[:].rearrange("p e g -> p (e g)")

                # ======== projection p = R x + t ========
                # ACT seeds dst = R_c2*z + t_c per (c,g) (per-partition scalar
                # APs force per-g ops); DVE then accumulates the X/Y terms for
                # all g at once with free-axis-broadcast R entries.
                if rep == 0 and it == 0:
                    emit_pretouch()
                p014 = p01[:].rearrange("p g (c n) -> p g c n", c=2)
                # z-row (c=2) first: the reciprocal then overlaps the x/y-row
                # accumulates instead of waiting for the whole projection
                for c in (2, 0, 1):
                    dst_all = p2t[:] if c == 2 else p014[:, :, c, :]
                    for g in range(G):
                        dst = p2t[:, g, :] if c == 2 else p01[:, g, c * NPT:(c + 1) * NPT]
                        s_z = R9f[:, (3 * c + 2) * G + g:(3 * c + 2) * G + g + 1]
                        s_b = PSf[:, (3 + c) * G + g:(3 + c) * G + g + 1]
                        if feat["use_act_proj"]:
                            nc.scalar.activation(dst, Zg[:, g, :], ACTF.Identity,
                                                 bias=s_b, scale=s_z)
                        else:
                            nc.vector.tensor_scalar(dst, Zg[:, g, :], s_z, s_b,
                                                    OP.mult, OP.add)
                    a0b = R9[:, 3 * c, :].unsqueeze(2).broadcast_to([P, G, NPT])
                    a1b = R9[:, 3 * c + 1, :].unsqueeze(2).broadcast_to([P, G, NPT])
                    nc.vector.tensor_tensor(paddS[c % 2][:], Xg, a0b, OP.mult)
                    nc.vector.tensor_tensor(paddS[2][:], Yg, a1b, OP.mult)
                    nc.vector.tensor_tensor(dst_all, dst_all, paddS[c % 2][:], OP.add)
                    nc.vector.tensor_tensor(dst_all, dst_all, paddS[2][:], OP.add)
                    if c == 2:
                        with nc.allow_low_precision(reason="iz consumed as bf16"):
                            nc.vector.reciprocal(
                                izt16[:].rearrange("p g n -> p (g n)"),
                                p2t[:].rearrange("p g n -> p (g n)"))

                def v4(t):
                    return t[:].rearrange("p g (s n) -> p g s n", s=2)

                izb = izt16[:].unsqueeze(2).broadcast_to([P, G, 2, NPT])
                nc.vector.tensor_tensor(v4(uvt), v4(p01), izb, OP.mult)

                # ======== E rows: e_sk = (uv_s * (-A2k) + A_sk) * iz ========
                # the last use_pool_e groups run on Pool (2-inst form) while
                # DVE handles the rest with the fused AFFINE_MUL custom op
                from concourse.dve_ops import AFFINE_MUL_REDUCE as _AMR
                n_pool_e = int(feat["use_pool_e"]) if feat["use_pool_split"] else 0
                for kk in range(3):
                    for s in range(2):
                        for g in range(G):
                            eo = E[kk][:, g, s * NPT:(s + 1) * NPT]
                            ei = uvt[:, g, s * NPT:(s + 1) * NPT]
                            s0 = nA2f[:, kk * G + g:kk * G + g + 1]
                            s1 = R9f[:, (3 * s + kk) * G + g:(3 * s + kk) * G + g + 1]
                            if g >= G - n_pool_e:
                                nc.gpsimd.tensor_scalar(ptmp[:], ei, s0, s1,
                                                        OP.mult, OP.add)
                                nc.gpsimd.tensor_tensor(eo, ptmp[:],
                                                        izt16[:, g, :], OP.mult)
                            elif feat["use_affine"]:
                                nc.vector._custom_dve(
                                    _AMR, out=eo, in0=ei,
                                    in1=izt16[:, g, :], s0=s0, s1=s1)
                            else:
                                tmp = sttbuf[:, 0:NPT].bitcast(BF16)[:, 0:NPT]
                                nc.vector.tensor_scalar(tmp, ei, s0, s1,
                                                        OP.mult, OP.add)
                                nc.vector.tensor_tensor(eo, tmp, izt16[:, g, :],
                                                        OP.mult)

                # ======== F rows: f_a = e_b x_c - e_c x_b (cyclic) ========
                # F2 runs on Pool (own scratch) while DVE does F0/F1; the
                # S pairs touching F2 come late enough to cover Pool's pace.
                for a in (2, 0, 1):
                    bq, cq = (a + 1) % 3, (a + 2) % 3
                    xc = XYZg[cq].unsqueeze(2).broadcast_to([P, G, 2, NPT])
                    xb = XYZg[bq].unsqueeze(2).broadcast_to([P, G, 2, NPT])
                    if a == 2 and feat["use_pool_split"]:
                        nc.gpsimd.tensor_tensor(v4(pfc1), v4(E[bq]), xc, OP.mult)
                        nc.gpsimd.tensor_tensor(v4(pfc2), v4(E[cq]), xb, OP.mult)
                        nc.gpsimd.tensor_tensor(Ft[a][:], pfc1[:], pfc2[:],
                                                OP.subtract)
                    else:
                        nc.vector.tensor_tensor(v4(fcr1), v4(E[bq]), xc, OP.mult)
                        nc.vector.tensor_tensor(v4(fcr2), v4(E[cq]), xb, OP.mult)
                        nc.vector.tensor_tensor(Ft[a][:], fcr1[:], fcr2[:],
                                                OP.subtract)
                # rres feeds only the sv products at the tail of the S stage;
                # emitted after F2 so Pool's in-order queue doesn't stall the
                # DVE F rows behind it
                reng = nc.gpsimd if feat["use_pool_split"] else nc.vector
                reng.tensor_tensor(rres[:], uvt[:], UV0[:], OP.subtract)

                # ======== S = sum J^T J, s = sum J^T r ========
                # diagonals: ACT Square+accumulate straight from the J tiles;
                # n_polar off-diagonal pairs via polarization
                #   S_ab = 0.5*[(Ja+Jb)^2 - Ja^2 - Jb^2]  (DVE add, ACT square)
                # remaining pairs: DVE bf16 mult + preadd + reduce.
                Jt = [Ft[0], Ft[1], Ft[2], E[0], E[1], E[2]]
                SQ5 = float(np.sqrt(0.5))
                n_polar = int(feat["n_polar"]) if feat["use_act_diag"] else 0
                pi = 0

                n_pool_pre = int(feat["use_pool_preadd"])
                # pipeline items [dst, pr, on_pool, pd, pd2, age]: src(k)
                # [DVE mult or bulk ACT square] -> preadd(k) at k+1 [every
                # n-th on Pool] -> half-preadd(k) at k+2 -> reduce(k) at k+3.
                # Deferral absorbs both Pool's slower preadds and ACT's
                # square latency; the 1x-rate tensor_reduce sees NPT/2 elems.
                mr_pending = []  # [dst, pr, on_pool, pd, pd2, pd3, age]
                mr_ctr = [0, 0, 0, 0]  # [pool-pd, dve-pd, pd2, pd3] counters
                H2, H4 = NPT // 2, NPT // 4

                def _mr_tick():
                    for item in mr_pending:
                        item[6] += 1
                    for item in list(mr_pending):
                        if item[5] is not None and item[6] >= 4:
                            nc.vector.tensor_reduce(item[0], item[5][:],
                                                    AX.X, OP.add)
                            mr_pending.remove(item)
                            break
                    for item in mr_pending:
                        if item[4] is not None and item[5] is None and item[6] >= 3:
                            pd3 = padd3[mr_ctr[3] % 3]
                            mr_ctr[3] += 1
                            nc.vector.tensor_tensor(pd3[:],
                                                    item[4][:, :, 0:H4],
                                                    item[4][:, :, H4:H2],
                                                    OP.add)
                            item[5] = pd3
                            break
                    for item in mr_pending:
                        if item[3] is not None and item[4] is None and item[6] >= 2:
                            pd2 = padd2[mr_ctr[2] % 3]
                            mr_ctr[2] += 1
                            nc.vector.tensor_tensor(pd2[:],
                                                    item[3][:, :, 0:H2],
                                                    item[3][:, :, H2:NPT],
                                                    OP.add)
                            item[4] = pd2
                            break
                    for item in mr_pending:
                        if item[3] is None and item[6] >= 1:
                            pr, on_pool = item[1], item[2]
                            if on_pool:
                                pd = ppadd[mr_ctr[0] % 3]
                                mr_ctr[0] += 1
                            else:
                                pd = paddS[mr_ctr[1] % 3]
                                mr_ctr[1] += 1
                            eng = nc.gpsimd if on_pool else nc.vector
                            eng.tensor_tensor(pd[:], pr[:, :, 0:NPT],
                                              pr[:, :, NPT:2 * NPT], OP.add)
                            item[3] = pd
                            break

                def mr_flush():
                    while mr_pending:
                        _mr_tick()

                def _mr_push(dst, pr, on_pool):
                    _mr_tick()
                    mr_pending.append([dst, pr, on_pool, None, None, 0])

                def mult_reduce(dst, in_a, in_b):
                    nonlocal pi
                    on_pool = (n_pool_pre > 0 and pi % 3 == 0
                               and pi // 3 < n_pool_pre)
                    pr = prodS[pi % 3]
                    pi += 1
                    nc.vector.tensor_tensor(pr[:], in_a[:], in_b[:], OP.mult)
                    _mr_push(dst, pr, on_pool)

                sqb = [pool.tile([P, G, 2 * NPT], BF16, name=f"sqb{i}")
                       for i in range(3)]
                sq_ctr = [0]

                def square_reduce(dst, in_a):
                    # diag entry: ONE bulk elementwise Square on ACT (no per-g
                    # accum_out — read-accumulator overhead makes that ~3x
                    # slower), then the shared preadd/reduce pipeline.
                    pr = sqb[sq_ctr[0] % 3]
                    sq_ctr[0] += 1
                    nc.scalar.activation(
                        pr[:].rearrange("p g n -> p (g n)"),
                        in_a[:].rearrange("p g n -> p (g n)"), ACTF.Square)
                    _mr_push(dst, pr, False)

                # index 2 (Ft[2]) is Pool-computed and lands last; order all
                # in-order engine streams so their Ft[2] consumers come last
                diag_order = [0, 1, 3, 4, 5, 2]
                offdiag = [(0, 1), (0, 3), (0, 4), (0, 5), (1, 3), (1, 4),
                           (1, 5), (3, 4), (3, 5), (4, 5),
                           (0, 2), (1, 2), (2, 3), (2, 4), (2, 5)]
                polar = []
                n_diag_bulk = int(feat.get("n_diag_bulk", 0))
                if feat["use_act_diag"]:
                    for di, a in enumerate(diag_order):
                        if di < n_diag_bulk:
                            square_reduce(SST[:, 7 * a, :], Jt[a])
                        else:
                            for g in range(G):
                                nc.scalar.activation(
                                    act_sink[:], Jt[a][:, g, :], ACTF.Square,
                                    accum_out=SST[:, 7 * a, g:g + 1])
                    n_pool_polar = int(feat.get("n_pool_polar", 0))
                    for (a, bq) in offdiag[:n_polar]:
                        ps = psum16[len(polar) % len(psum16)]
                        # last k pairs' sums on Pool: their ACT squares come
                        # late in ACT's queue, covering Pool's slower add
                        peng = (nc.gpsimd
                                if len(polar) >= n_polar - n_pool_polar
                                else nc.vector)
                        peng.tensor_tensor(ps[:], Jt[a][:], Jt[bq][:],
                                           OP.add)
                        qrow = len(polar)
                        for g in range(G):
                            nc.scalar.activation(
                                act_sink[:], ps[:, g, :], ACTF.Square,
                                scale=SQ5,
                                accum_out=Qpol[:, qrow, g:g + 1])
                        polar.append((a, bq, qrow))
                else:
                    for a in range(6):
                        mult_reduce(SST[:, 7 * a, :], Jt[a], Jt[a])
                for (a, bq) in offdiag[n_polar:]:
                    mult_reduce(SST[:, 6 * a + bq, :], Jt[a], Jt[bq])
                # sv: first n_sv_act entries go product-on-DVE + Copy-accum
                # reduce on ACT (balancing the engines); rest are mult_reduce
                n_sv_act = int(feat.get("n_sv_act", 0))
                for ai, a in enumerate(diag_order[:n_sv_act]):
                    pr = svpr[ai % 3]
                    nc.vector.tensor_tensor(pr[:], Jt[a][:], rres[:], OP.mult)
                    for g in range(G):
                        nc.scalar.activation(act_sink[:], pr[:, g, :],
                                             ACTF.Copy,
                                             accum_out=sv[:, a, g:g + 1])
                for a in diag_order[n_sv_act:]:
                    mult_reduce(sv[:, a, :], Jt[a], rres)
                mr_flush()
                # combine polarized pairs: S_ab = Q' - 0.5*Da - 0.5*Db
                if polar:
                    Dh = inv_ws["cof"]  # scratch [P, 9, G], unused until _inv3
                    for a in range(6):
                        nc.vector.tensor_scalar(Dh[:, a, :], SST[:, 7 * a, :],
                                                0.5, None, OP.mult)
                    for (a, bq, qrow) in polar:
                        nc.vector.tensor_tensor(tmpg[:], Dh[:, a, :], Dh[:, bq, :],
                                                OP.add)
                        nc.vector.tensor_tensor(SST[:, 6 * a + bq, :],
                                                Qpol[:, qrow, :], tmpg[:],
                                                OP.subtract)
                # mirror lower triangle: rows 7a+d -> 7a+6d, a<6-d
                # (on DVE: by this point DVE has already waited on the ACT and
                # Pool clocks, so these carry no extra sem waits — the walrus
                # build allows at most one wait per instruction)
                for d in range(1, 6):
                    n = 6 - d
                    nc.vector.tensor_copy(SST[:, 6 * d:6 * d + 7 * (n - 1) + 1:7, :],
                                          SST[:, d:d + 7 * (n - 1) + 1:7, :])

                # ======== congruence H = W^T S W (W = blockdiag(Jr, R^T)) ========
                SS4 = SST[:].rearrange("p (a l) g -> p a l g", a=6)
                Srr = SS4[:, 0:3, 0:3, :]
                Srt = SS4[:, 0:3, 3:6, :]
                Stt = SS4[:, 3:6, 3:6, :]
                # Htt chain (UV block, Q9, M9) on Pool; Hrr chain + inverses
                # on DVE; gr3/gt3 fill DVE's wait on Pool's Q9/M9.
                A6 = SS4[:, 0:6, 3:6, :]                    # [p, 6, 3, g]
                # UV = [Srt; Stt] @ R^T: B[l,b] = R[b,l] via transposed view
                RT4 = _stack3(R9).transpose([0, 2, 1, 3])
                prodP6 = prodP[:].rearrange(
                    "p a b l g -> p (a b) l g")[:, 0:6, :, :]   # [P, 6, 3, G]
                _matmul3_nored(nc.gpsimd, prodP6, UVP[:], A6, RT4, nrow=6)
                U9v = UVP[:, 0:9, :]
                V9v = UVP[:, 9:18, :]
                prodP3 = prodP[:, 0, :, :, :]
                _matmul3_nored(nc.gpsimd, prodP3, Q9[:], _stack3(J9),
                               U9v.rearrange("p (l b) g -> p l b g", l=3),
                               transA=True)
                _matmul3_nored(nc.gpsimd, prodP3, M9[:], _stack3(R9),
                               V9v.rearrange("p (l b) g -> p l b g", l=3))
                nc.gpsimd.tensor_scalar(M9[:, 0:9:4, :], M9[:, 0:9:4, :],
                                        float(damp), None, OP.add)
                _matmul3(nc, prod, T1, Srr, J9[:])
                _matmul3(nc, prod, P9, _stack3(J9), T1[:], transA=True)  # Hrr

                # ======== damped Schur solve ========
                nc.vector.tensor_scalar(P9[:, 0:9:4, :], P9[:, 0:9:4, :],
                                        float(damp), None, OP.add)
                _inv3(nc, inv_ws, P9, Pinv, G)
                _matvec3(nc, prod, gr3[:], _stack3(J9), sv[:, 0:3, :], transA=True)
                _matvec3(nc, prod, gt3[:], _stack3(R9), sv[:, 3:6, :])
                _matmul3(nc, prod, QtPi, _stack3(Q9), Pinv[:], transA=True)
                _matmul3(nc, prod, U9, _stack3(QtPi), Q9[:], sub_from=None)
                nc.vector.tensor_tensor(M9[:], M9[:], U9[:], OP.subtract)
                _inv3(nc, inv_ws, M9, Minv, G)
                _matvec3(nc, prod, rhs_t[:], _stack3(QtPi), gr3[:], sub_from=gt3[:])
                _matvec3(nc, prod, dt3[:], _stack3(Minv), rhs_t[:])
                _matvec3(nc, prod, rhs_r[:], _stack3(Q9), dt3[:], sub_from=gr3[:])
                _matvec3(nc, prod, dr3[:], _stack3(Pinv), rhs_r[:])

                # pose update: rot += dr' (sign-flipped), t -= dt
                nc.vector.tensor_tensor(PS[:, 0:3, :], PS[:, 0:3, :], dr3[:], OP.add)
                nc.vector.tensor_tensor(PS[:, 3:6, :], PS[:, 3:6, :], dt3[:], OP.subtract)

                if rep == 0 and it == 0 and dbg_requests:
                    local = dict(R9=R9, J9=J9, nA2=nA2, p01=p01,
                                 p2t=p2t, izt=izt, uvt=uvt, rres=rres, SST=SST,
                                 sv=sv, Hrr=P9, Q9=Q9, Htt=M9, gr3=gr3, gt3=gt3,
                                 Pinv=Pinv, Minv=Minv, QtPi=QtPi, dt3=dt3, dr3=dr3,
                                 th=th, sth=sth, cth=cth, k3=k3,
                                 E0=E[0], E1=E[1], E2=E[2],
                                 F0=Ft[0], F1=Ft[1], F2=Ft[2])
                    for nm in dbg_requests:
                        t = local[nm]
                        ap = t[:]
                        fshape = [P, ap.free_size()]
                        dram = nc.declare_dram_parameter(f"dbg_{nm}", fshape, F32,
                                                         isOutput=True)
                        flatap = ap
                        while len(flatap.shape) > 2:
                            flatap = flatap.rearrange(
                                "p " + " ".join(f"d{i}" for i in range(len(flatap.shape) - 1))
                                + " -> p (" + " ".join(f"d{i}" for i in range(len(flatap.shape) - 1)) + ")")
                        if t[:].dtype != F32:
                            cv = pool.tile(fshape, F32, name=f"dbgc_{nm}")
                            nc.vector.tensor_copy(cv[:], flatap)
                            flatap = cv[:]
                        nc.sync.dma_start(out=dram[:], in_=flatap)
                        dbg_tiles[nm] = fshape

              for fz in range(frozen):
                # ==== pass B: reproject at current pose; J, W, H^-1 frozen ====
                # Fixed point solves J(p0)^T r(p*) = 0; H only preconditions,
                # so the E/F tiles, congruence W(p0) and Schur pieces are all
                # reused from the full iteration (study: rel 2.1e-3 vs 8-iter
                # reference for 1 full + 1 frozen, 10x inside the 2e-2 gate).
                #
                # Rotation via the right-Jacobian identity instead of a full
                # rodrigues recompute: R(r0+dr) = R(r0) exp([Jr(r0) dr]x),
                # 2nd-order exp; error O(|dr|^3) ~ 2e-5 (validated: 2.075e-3
                # vs 2.098e-3 with exact rodrigues).
                w3 = k3   # scratch [P,3,G], free in this pass
                M9b = U9  # scratch [P,9,G], free in this pass
                _matvec3(nc, prod, w3[:], _stack3(J9), dr3[:])
                Mb4 = _stack3(M9b)
                wl = w3[:].unsqueeze(2).broadcast_to([P, 3, 3, G])
                wc = w3[:].unsqueeze(1).broadcast_to([P, 3, 3, G])
                nc.vector.tensor_tensor(Mb4, wl, wc, OP.mult)      # w_l w_b
                nc.vector.tensor_scalar(M9b[:], M9b[:], 0.5, None, OP.mult)
                Mdiag = M9b[:, 0:9:4, :]
                nc.vector.tensor_reduce(th2[:], Mdiag.transpose([0, 2, 1]),
                                        AX.X, OP.add)              # 0.5|w|^2
                nc.vector.tensor_scalar(tmpg[:], th2[:], -1.0, 1.0,
                                        OP.mult, OP.add)           # 1 - 0.5|w|^2
                tb = tmpg[:].unsqueeze(1).broadcast_to([P, 3, G])
                nc.vector.tensor_tensor(Mdiag, Mdiag, tb, OP.add)
                for (row, wi, op) in ((1, 2, OP.subtract), (2, 1, OP.add),
                                      (3, 2, OP.add), (5, 0, OP.subtract),
                                      (6, 1, OP.subtract), (7, 0, OP.add)):
                    nc.vector.tensor_tensor(M9b[:, row:row + 1, :],
                                            M9b[:, row:row + 1, :],
                                            w3[:, wi:wi + 1, :], op)
                _matmul3(nc, prod, R9b, _stack3(R9), M9b[:])
                # projection at current pose (normalized coords: A=R(p), b=t);
                # ACT (idle in this pass) does the z-row affine seed, DVE the
                # all-g broadcast X/Y accumulates
                PSfb = PS[:].rearrange("p e g -> p (e g)")
                p014b = p01[:].rearrange("p g (c n) -> p g c n", c=2)
                # z-row seeds on DVE (cheap 4x TSP) so its accumulates and
                # the reciprocal never wait on ACT's seed chain; x/y rows
                # seed on ACT in parallel
                for c in (2, 0, 1):
                    dst_all = p2t[:] if c == 2 else p014b[:, :, c, :]
                    for g in range(G):
                        dst = p2t[:, g, :] if c == 2 else p01[:, g, c * NPT:(c + 1) * NPT]
                        s_z = R9bf[:, (3 * c + 2) * G + g:(3 * c + 2) * G + g + 1]
                        s_b = PSfb[:, (3 + c) * G + g:(3 + c) * G + g + 1]
                        if c == 2:
                            nc.vector.tensor_scalar(dst, Zg[:, g, :], s_z, s_b,
                                                    OP.mult, OP.add)
                        else:
                            nc.scalar.activation(dst, Zg[:, g, :], ACTF.Identity,
                                                 bias=s_b, scale=s_z)
                    a0b = R9b[:, 3 * c, :].unsqueeze(2).broadcast_to([P, G, NPT])
                    a1b = R9b[:, 3 * c + 1, :].unsqueeze(2).broadcast_to([P, G, NPT])
                    nc.vector.tensor_tensor(paddS[c % 2][:], Xg, a0b, OP.mult)
                    nc.vector.tensor_tensor(paddS[2][:], Yg, a1b, OP.mult)
                    nc.vector.tensor_tensor(dst_all, dst_all, paddS[c % 2][:], OP.add)
                    nc.vector.tensor_tensor(dst_all, dst_all, paddS[2][:], OP.add)
                    if c == 2:
                        with nc.allow_low_precision(reason="iz consumed as bf16"):
                            nc.vector.reciprocal(
                                izt16[:].rearrange("p g n -> p (g n)"),
                                p2t[:].rearrange("p g n -> p (g n)"))
                izb2 = izt16[:].unsqueeze(2).broadcast_to([P, G, 2, NPT])
                nc.vector.tensor_tensor(v4(uvt), v4(p01), izb2, OP.mult)
                # rres on DVE here: it gates every sv product, and Pool would
                # take ~4x longer
                nc.vector.tensor_tensor(rres[:], uvt[:], UV0[:], OP.subtract)
                # sv_a = sum_pts J_a(p0) * r — first n_sv_act_b entries reduce
                # on ACT (idle after the projection seeds); rest on DVE with
                # alternating Pool/DVE preadds, reduce deferred one step
                nsab = int(feat.get("n_sv_act_b", 0))
                sv_order = (0, 1, 3, 4, 5, 2)
                for ai, a in enumerate(sv_order[:nsab]):
                    pr = svpr[ai % 3]
                    nc.vector.tensor_tensor(pr[:], Jt[a][:], rres[:], OP.mult)
                    for g in range(G):
                        nc.scalar.activation(act_sink[:], pr[:, g, :],
                                             ACTF.Copy,
                                             accum_out=sv[:, a, g:g + 1])
                for a in sv_order[nsab:]:
                    mult_reduce(sv[:, a, :], Jt[a], rres)
                mr_flush()
                # g = W^T s and the frozen Schur solve
                _matvec3(nc, prod, gr3[:], _stack3(J9), sv[:, 0:3, :], transA=True)
                _matvec3(nc, prod, gt3[:], _stack3(R9), sv[:, 3:6, :])
                _matvec3(nc, prod, rhs_t[:], _stack3(QtPi), gr3[:], sub_from=gt3[:])
                _matvec3(nc, prod, dt3[:], _stack3(Minv), rhs_t[:])
                _matvec3(nc, prod, rhs_r[:], _stack3(Q9), dt3[:], sub_from=gr3[:])
                _matvec3(nc, prod, dr3[:], _stack3(Pinv), rhs_r[:])
                nc.vector.tensor_tensor(PS[:, 0:3, :], PS[:, 0:3, :], dr3[:], OP.add)
                nc.vector.tensor_tensor(PS[:, 3:6, :], PS[:, 3:6, :], dt3[:], OP.subtract)

            nc.sync.dma_start(out=out_d[:], in_=PS[:].rearrange("p e g -> p (e g)"))
            # DMA-completion observability chain (see _patch_tail_drain)
            jrd = pool.tile([P, 6], F32)
            jrd2 = pool.tile([P, 6], F32)
            nc.sync.dma_start(out=jrd[:], in_=out_d[:, 0:6])
            nc.vector.tensor_copy(jrd2[:], jrd[:])

    from concourse.library_overlay import lower_extended_insts
    lower_extended_insts(nc)
    # Split multi-sem waits into InstEventSemaphore pairs: this walrus build
    # rejects >1 raw sem wait per instruction, and the cross-engine pipeline
    # (DVE/Pool/ACT) legitimately produces a few two-wait joins.
    import bass_rust as _bass_rust
    _bass_rust.generate_event_semaphores(nc)
    return nc


# ---------------------------------------------------------------------------
# host-side sharding + execution
# ---------------------------------------------------------------------------

def _normalize_uv(pts2d, K):
    """Pixel -> normalized camera coords: (uv - c) / f. The device kernel
    works K-free (A=R, b=t); H and g scale by f^2, so the damped GN step is
    unchanged."""
    K = np.asarray(K, np.float64)
    f = np.array([K[0, 0], K[1, 1]])
    c = np.array([K[0, 2], K[1, 2]])
    return ((np.asarray(pts2d, np.float64) - c) / f).astype(np.float32)


def _shard_core(pts2d_c, pts3d_c, init_pose_c, G):
    xyz = pts3d_c.reshape(G, P, NPT, 3).transpose(3, 1, 0, 2).reshape(3, P, G * NPT)
    uv0 = pts2d_c.reshape(G, P, NPT, 2).transpose(1, 0, 3, 2).reshape(P, G * 2 * NPT)
    pose0 = init_pose_c.reshape(G, P, 6).transpose(1, 2, 0).reshape(P, 6 * G)
    inp16 = np.concatenate([xyz[0], xyz[1], xyz[2], uv0], axis=1)
    return {"inp": np.ascontiguousarray(inp16).astype(ml_dtypes.bfloat16),
            "pose0": np.ascontiguousarray(pose0, np.float32)}


def _unshard_core(pose_out, G):
    return pose_out.reshape(P, 6, G).transpose(2, 0, 1).reshape(G * P, 6)


def kernel(pts2d, pts3d, K, init_pose):
    pts2d = np.asarray(pts2d, np.float32)
    pts3d = np.asarray(pts3d, np.float32)
    K = np.asarray(K, np.float32)
    init_pose = np.asarray(init_pose, np.float32)

    batch = pts3d.shape[0]
    bpc = batch // NCORES
    G = bpc // P

    nc = build_nc(K, G=G)
    pts2d_n = _normalize_uv(pts2d, K)
    in_maps = [
        _shard_core(pts2d_n[c * bpc:(c + 1) * bpc], pts3d[c * bpc:(c + 1) * bpc],
                    init_pose[c * bpc:(c + 1) * bpc], G)
        for c in range(NCORES)
    ]
    res = run_bass_kernel_spmd(nc, in_maps, list(range(NCORES)))
    outs = [_unshard_core(res.results[c]["pose_out"], G) for c in range(NCORES)]
    return np.concatenate(outs, axis=0).astype(np.float32)


if __name__ == "__main__":
    rng = np.random.default_rng(0)
    Km = np.array([[800.0, 0, 320.0], [0, 800.0, 240.0], [0, 0, 1.0]], np.float32)
    pts3d = rng.standard_normal((8192, 128, 3)).astype(np.float32)
    pose = np.concatenate([0.2 * rng.standard_normal((8192, 3)),
                           0.3 * rng.standard_normal((8192, 2)),
                           6 + 0.5 * rng.random((8192, 1))], axis=1).astype(np.float32)
    pts2d = rng.standard_normal((8192, 128, 2)).astype(np.float32) * 100
    out = kernel(pts2d, pts3d, Km, pose)
    print(out.shape, out.dtype, np.isfinite(out).mean())



# revision 67
# speedup vs baseline: 1.0192x; 1.0192x over previous
"""Trainium2 Bass kernel: batched PnP refinement (8192 instances).

Sharding: data-parallel over instances, 1024 per core x 8 cores.
Per-core layout: instances -> 8 groups x 128 partitions; points (128) on the
free axis. Inputs are packed bf16 in normalized camera coordinates
(uv0 -> (uv0-c)/f on host): A=R and b=t on device, and since both H and g
scale by f^2 the damped GN step is unchanged (validated to 2e-6).

Algorithm: NITER=1 full damped-GN iteration (build J, S=J^T J, the
W-congruence and the Schur-factored H^-1 pieces, take the first step),
then NFROZEN=1 cheap pass that only reprojects at the updated pose and
solves with the FROZEN J/W/H^-1: the frozen fixed point solves
J(p0)^T r(p*)=0, H only preconditions. f64 study: 2.1e-3 rel vs the
8-iteration reference (vs 3e-4 for two full iterations), ~7x inside the
2e-2 gate with bf16 noise included. The frozen pass rebuilds R via the
right-Jacobian exp identity R(r0+dr)=R(r0)exp([Jr dr]x) (2nd-order exp).

Engine split (tuned against the TimelineSim cost model, then HW):
  - DVE: rodrigues, projection X/Y accumulates (all-g broadcast TT),
    E rows (AFFINE_MUL custom op), F0/F1, S products, the staged
    preadd/halve/halve/reduce pipeline (reduce runs at 1x so it only ever
    sees NPT/4 elements), congruence Hrr chain, 3x3 inverses, Schur solve.
  - Pool (GpSimd): F2 triple, residual, a share of S preadds and polar
    sums, the Htt congruence chain (reduce-free 3x3 matmuls).
  - ACT: projection z-row seeds, S diagonal via Square+accum (2 entries
    as bulk Square into the DVE reduce pipeline), n_polar off-diagonal
    entries via polarization S_ab = 0.5[(Ja+Jb)^2 - Ja^2 - Jb^2].
All cross-stage reduce consumers are pipelined 3 steps behind their
producers so DVE never stalls on Pool/ACT pace. The walrus build allows
at most one raw sem wait per instruction; generate_event_semaphores()
splits the multi-engine joins, and per-engine pre-touches of the input
DMA keep later cross-engine waits single. bf16 error in J/residual tiles
acts as zero-mean per-point noise on a least-squares fit over 256 rows;
the pose shift it induces is O(1e-4), far below the gate.
"""
import sys

if "/opt/trn_rl_repo" not in sys.path:
    sys.path.insert(0, "/opt/trn_rl_repo")

import numpy as np
import ml_dtypes

import concourse.bass as bass
import concourse.mybir as mybir
from concourse import tile
from concourse.bass_utils import run_bass_kernel_spmd

F32 = mybir.dt.float32
BF16 = mybir.dt.bfloat16
AX = mybir.AxisListType
OP = mybir.AluOpType
ACTF = mybir.ActivationFunctionType

# sin/cos polynomial coefficients (odd/even powers) — fallback path only
SIN_C = [0.9999999959708131, -0.16666665042663348, 0.008333314505395609,
         -0.0001984031090520505, 2.753228838784914e-06, -2.4701576164777272e-08,
         1.3533152847536427e-10]
COS_C = [0.9999999922740526, -0.49999991767336033, 0.041666524297492756,
         -0.0013887970070279262, 2.477341646686846e-05, -2.7113293396156204e-07,
         1.7368828593492213e-09]

P = 128      # partitions (instances per group)
NPT = 128    # points per instance
NCORES = 8
NITER = 1    # full LM iterations (J, S, H built + first step)
NFROZEN = 1  # cheap iterations: reproject + g = J^T r with frozen J, H^-1
DAMP = 1e-4


def _lincomb(nc, stt, out, terms):
    """out[:, rows, :] = sum coeff * ap  with compile-time float coeffs."""
    terms = [(float(c), ap) for c, ap in terms if float(c) != 0.0]
    if not terms:
        nc.vector.memset(out, 0.0)
        return
    c0, a0 = terms[0]
    nc.vector.tensor_scalar(out, a0, c0, None, OP.mult)
    for c, ap in terms[1:]:
        stt(out, ap, c, out, OP.mult, OP.add)


def _stack3(t):
    """[128, 9, G] stack -> 4D view [128, 3, 3, G] (row-major 3x3)."""
    return t[:].rearrange("p (a b) g -> p a b g", a=3)


def _matmul3(nc, prod, out9, a_ap4, b_ap, transA=False, transB=False,
             sub_from=None, eng=None):
    """out9[a,b] = sum_l A[a,l] * B[l,b] for stacked 3x3 per-instance mats.

    b_ap: [128, 9, G] AP. Per-column form (the ISA allows at most 3 free AP
    dims, so the fully batched [p,b,a,l,g] variant cannot be encoded).
    """
    v = eng or nc.vector
    G = b_ap.shape[-1]
    if transA:
        a_ap4 = a_ap4.transpose([0, 2, 1, 3])
    b4 = b_ap.rearrange("p (a b) g -> p a b g", a=3)
    out4 = _stack3(out9)
    for b in range(3):
        col = b4[:, b, :, :] if transB else b4[:, :, b, :]  # [128, 3, G] over l
        col = col.unsqueeze(1).broadcast_to([P, 3, 3, G])
        v.tensor_tensor(prod[:, 0, :, :, :], a_ap4, col, OP.mult)
        red_in = prod[:, 0, :, :, :].transpose([0, 1, 3, 2])  # (a, g, l) reduce l
        v.tensor_reduce(out4[:, :, b, :], red_in, AX.X, OP.add)
    if sub_from is not None:
        v.tensor_tensor(out9[:], sub_from[:], out9[:], OP.subtract)


def _matmul3_nored(eng, tmp, out9, a_ap4, b4, transA=False, nrow=3):
    """Reduce-free stacked matmul for engines without free-axis reduce
    (Pool): out[a,b] = sum_l A[a,l]*B[l,b] as 3 broadcast mults + 2 adds.

    a_ap4: [128, nrow, 3, G]; b4: [128, 3(l), 3(b), G] view (transpose views
    allowed); out9: [128, nrow*3, G]; tmp: [128, nrow, 3, G] scratch.
    """
    G = b4.shape[-1]
    if transA:
        a_ap4 = a_ap4.transpose([0, 2, 1, 3])
    out4 = out9.rearrange("p (a b) g -> p a b g", a=nrow)
    for l in range(3):
        al = a_ap4[:, :, l, :].unsqueeze(2).broadcast_to([P, nrow, 3, G])
        bl = b4[:, l, :, :].unsqueeze(1).broadcast_to([P, nrow, 3, G])
        if l == 0:
            eng.tensor_tensor(out4, al, bl, OP.mult)
        else:
            eng.tensor_tensor(tmp, al, bl, OP.mult)
            eng.tensor_tensor(out4, out4, tmp, OP.add)


def _matvec3(nc, prod3, out3, a_ap4, x3, transA=False, sub_from=None, eng=None):
    """out3[i] = sum_k A[i,k] x[k]; x3, out3: [128, 3, G]; prod3: [128,3,3,3,G]."""
    v = eng or nc.vector
    G = x3.shape[-1]
    if transA:
        a_ap4 = a_ap4.transpose([0, 2, 1, 3])
    xb = x3.unsqueeze(1).broadcast_to([P, 3, 3, G])
    p3v = prod3[:, 0, :, :, :]
    v.tensor_tensor(p3v, a_ap4, xb, OP.mult)
    red_in = p3v.transpose([0, 1, 3, 2])
    v.tensor_reduce(out3, red_in, AX.X, OP.add)
    if sub_from is not None:
        v.tensor_tensor(out3, sub_from, out3, OP.subtract)


def _inv3(nc, ws, src9, out9, G):
    """Explicit 3x3 inverse of stacked mats via adjugate (6x6 replication)."""
    mw, cof, t2 = ws["mw"], ws["cof"], ws["t2"]
    det, idet, p3 = ws["det"], ws["idet"], ws["p3"]
    mwf = mw[:].rearrange("p (a b) g -> p a b g", a=6)
    src4 = _stack3(src9)
    for (ra, rb) in ((0, 0), (0, 3), (3, 0), (3, 3)):
        nc.vector.tensor_copy(mwf[:, ra:ra + 3, rb:rb + 3, :], src4)

    def g(da, db):
        return mwf[:, da:da + 3, db:db + 3, :]

    nc.vector.tensor_tensor(_stack3(cof), g(1, 1), g(2, 2), OP.mult)
    nc.vector.tensor_tensor(_stack3(t2), g(1, 2), g(2, 1), OP.mult)
    nc.vector.tensor_tensor(cof[:], cof[:], t2[:], OP.subtract)
    nc.vector.tensor_tensor(p3[:], src9[:, 0:3, :], cof[:, 0:3, :], OP.mult)
    nc.vector.tensor_reduce(det[:], p3[:].transpose([0, 2, 1]), AX.X, OP.add)
    nc.vector.reciprocal(idet[:], det[:])
    cofT = cof[:].rearrange("p (b a) g -> p b a g", b=3).transpose([0, 2, 1, 3])
    ib = idet[:].unsqueeze(1).unsqueeze(1).broadcast_to([P, 3, 3, G])
    nc.vector.tensor_tensor(_stack3(out9), cofT, ib, OP.mult)


FEATURES = dict(use_stt=True, use_affine=True, use_act_sin=True,
                use_act_diag=True, use_act_proj=True, use_pool_split=True,
                use_pool_preadd=6, use_pool_e=0, use_recip_approx=False,
                n_polar=7, n_sv_act=1, n_sv_act_b=2, n_diag_bulk=2,
                n_pool_polar=3)


def _patch_tail_drain():
    """Replace TileContext's tail drain with a wait-free variant.

    The walrus build here cannot encode the tail Drain's raw multi-sem waits
    ("Too many sync wait commands"). The kernel instead makes every DMA
    completion observable by the DVE engine (DRAM read-back chain emitted in
    build_nc), after which the raw waits on the drain are redundant.
    """
    from concourse import tile as _tile
    if getattr(_tile.TileContext, "_ant_tail_patched", False):
        return

    def _drain_and_barrier(self, tick_clock, wait_clock):
        self.nc.sync.drain()
        self.nc.all_engine_barrier()
        assert self.sems is not None
        popped = self.nc._tile_sem_poison_stack.pop()
        assert popped is self._sem_poison
        self.nc.clear_and_free_semaphores(list(self.sems.allocated().values()))
        self.nc.all_engine_barrier()

    _tile.TileContext._drain_and_barrier = _drain_and_barrier
    _tile.TileContext._ant_tail_patched = True


def build_nc(K, G=8, niter=NITER, damp=DAMP, debug_names=(), features=None,
             reps=1, frozen=NFROZEN):
    """Build the single-core Bass program (SPMD-replicated across cores).

    reps > 1 re-runs the whole refinement (pose reload + niter iterations)
    that many times back-to-back; used by test.py to measure per-kernel HW
    execution time as the marginal cost of extra reps, cancelling the
    ~60 ms per-dispatch axon/PJRT overhead.
    """
    feat = dict(FEATURES)
    if features:
        feat.update(features)
    _patch_tail_drain()
    from concourse.dve_ops import AFFINE_MUL_REDUCE
    from concourse.dve_ops import TENSOR_TENSOR_REDUCE as TTR

    K = np.asarray(K, np.float64)
    NI = G * NPT

    nc = bass.Bass(use_seq_codegen=feat.get("use_seq", False))
    # inputs: bf16 packed [X | Y | Z | UV0]; f32 pose
    NIN16 = 3 * NI + G * 2 * NPT
    inp_d = nc.declare_dram_parameter("inp", [P, NIN16], BF16, isOutput=False)
    pose_d = nc.declare_dram_parameter("pose0", [P, 6 * G], F32, isOutput=False)
    out_d = nc.declare_dram_parameter("pose_out", [P, 6 * G], F32, isOutput=True)
    dbg_requests = list(debug_names)
    dbg_tiles = {}

    with tile.TileContext(nc) as tc:
        with tc.tile_pool(name="main", bufs=1) as pool:
            # ---------------- persistent data ----------------
            inp_t = pool.tile([P, NIN16], BF16)
            # pose stacks, entry-major; double-buffered across timing reps so
            # rep k+1's reload DMA hides under rep k's compute
            PSt = [pool.tile([P, 6, G], F32, name=f"PS{i}") for i in range(2)]
            PS = PSt[0]
            # pose DMA first: it is tiny and gates the rodrigues stage, which
            # then overlaps the big packed-input DMA
            nc.sync.dma_start(out=PS[:].rearrange("p e g -> p (e g)"),
                              in_=pose_d[:])
            nc.sync.dma_start(out=inp_t[:], in_=inp_d[:])
            Xt = inp_t[:, 0:NI]
            Yt = inp_t[:, NI:2 * NI]
            Zt = inp_t[:, 2 * NI:3 * NI]
            UV0 = inp_t[:, 3 * NI:NIN16].rearrange("p (g n) -> p g n", g=G)

            I32 = mybir.dt.int32
            c5f = pool.tile([P, G], I32)
            nc.vector.memset(c5f[:], 0x5F3759DF)

            # Per-engine pre-touch of the bf16 input DMA: the walrus build
            # allows at most ONE sem wait per instruction, and per-engine
            # clock waits are monotonic, so each engine absorbs the input
            # DMA semaphore once; later reads of inp_t then never combine a
            # DMA wait with a cross-engine clock wait. Emitted lazily (just
            # before the first projection) so the rodrigues stage — which
            # reads only the small pose DMA — overlaps the big input DMA.
            pt_d = pool.tile([P, 1], BF16, name="pt_d")
            pt_p = pool.tile([P, 1], BF16, name="pt_p")
            pt_a = pool.tile([P, 1], BF16, name="pt_a")

            def emit_pretouch():
                nc.vector.tensor_copy(pt_d[:], inp_t[:, 0:1])
                nc.gpsimd.tensor_copy(pt_p[:], inp_t[:, 0:1])
                nc.scalar.activation(pt_a[:], inp_t[:, 0:1], ACTF.Copy)

            # per-point working tiles (bf16)
            p01 = pool.tile([P, G, 2 * NPT], BF16)
            p2t = pool.tile([P, G, NPT], BF16)
            izt = pool.tile([P, G, NPT], F32)
            izt16 = pool.tile([P, G, NPT], BF16)
            rsc = pool.tile([P, G, NPT], F32)   # reciprocal scratch
            uvt = pool.tile([P, G, 2 * NPT], BF16)
            rres = pool.tile([P, G, 2 * NPT], BF16)
            E = [pool.tile([P, G, 2 * NPT], BF16, name=f"E{i}") for i in range(3)]
            Ft = [pool.tile([P, G, 2 * NPT], BF16, name=f"Ft{i}") for i in range(3)]
            fcr1 = pool.tile([P, G, 2 * NPT], BF16)
            fcr2 = pool.tile([P, G, 2 * NPT], BF16)
            pfc1 = pool.tile([P, G, 2 * NPT], BF16)  # Pool-engine F scratch
            pfc2 = pool.tile([P, G, 2 * NPT], BF16)
            ptmp = pool.tile([P, NPT], BF16)         # Pool-engine E scratch
            # rotating product/preadd buffers (DVE mult+preadd+reduce pipeline)
            prodS = [pool.tile([P, G, 2 * NPT], BF16, name=f"prodS{i}")
                     for i in range(3)]
            paddS = [pool.tile([P, G, NPT], BF16, name=f"paddS{i}")
                     for i in range(3)]
            ppadd = [pool.tile([P, G, NPT], BF16, name=f"ppadd{i}")
                     for i in range(3)]
            padd2 = [pool.tile([P, G, NPT // 2], BF16, name=f"padd2_{i}")
                     for i in range(3)]
            padd3 = [pool.tile([P, G, NPT // 4], BF16, name=f"padd3_{i}")
                     for i in range(3)]
            # polarized-pair sum buffers (DVE add -> ACT square+accum); one per
            # polarized pair so the DVE adds never stall on ACT's reads
            psum16 = [pool.tile([P, G, 2 * NPT], BF16, name=f"psum16_{i}")
                      for i in range(max(1, int(FEATURES["n_polar"])))]
            act_sink = pool.tile([P, 2 * NPT], BF16)
            # dedicated product buffers for ACT-reduced sv entries (ACT reads
            # them late in its queue; prodS rotation would stall DVE)
            svpr = [pool.tile([P, G, 2 * NPT], BF16, name=f"svpr{i}")
                    for i in range(3)]
            Qpol = pool.tile([P, 16, G], F32)   # polarized Q_ab accumulators
            sttbuf = pool.tile([P, G * 2 * NPT], F32)

            # per-instance stacks [128, n, G]
            sq3 = pool.tile([P, 3, G], F32)
            th2 = pool.tile([P, G], F32)
            th = pool.tile([P, G], F32)
            ith = pool.tile([P, G], F32)
            sth = pool.tile([P, G], F32)
            s2h = pool.tile([P, G], F32)
            cth = pool.tile([P, G], F32)
            omc = pool.tile([P, G], F32)
            alf = pool.tile([P, G], F32)
            bet = pool.tile([P, G], F32)
            omb = pool.tile([P, G], F32)
            tmpg = pool.tile([P, G], F32)
            k3 = pool.tile([P, 3, G], F32)
            kkd = pool.tile([P, 3, G], F32)
            kko = pool.tile([P, 3, G], F32)
            sk = pool.tile([P, 3, G], F32)
            ak = pool.tile([P, 3, G], F32)
            okkd = pool.tile([P, 3, G], F32)
            okko = pool.tile([P, 3, G], F32)
            bkkd = pool.tile([P, 3, G], F32)
            bkko = pool.tile([P, 3, G], F32)
            R9 = pool.tile([P, 9, G], F32)
            R9b = pool.tile([P, 9, G], F32)  # pass-B rotation (R9 stays frozen)
            J9 = pool.tile([P, 9, G], F32)
            nA2 = pool.tile([P, 3, G], F32)
            SST = pool.tile([P, 36, G], F32)
            sv = pool.tile([P, 6, G], F32)
            prod = pool.tile([P, 3, 3, 3, G], F32)
            T1 = pool.tile([P, 9, G], F32)
            Hrr = pool.tile([P, 9, G], F32)
            U9 = pool.tile([P, 9, G], F32)
            Q9 = pool.tile([P, 9, G], F32)
            V9 = pool.tile([P, 9, G], F32)
            Htt = pool.tile([P, 9, G], F32)
            gr3 = pool.tile([P, 3, G], F32)
            gt3 = pool.tile([P, 3, G], F32)
            P9 = pool.tile([P, 9, G], F32)
            M9 = pool.tile([P, 9, G], F32)
            inv_ws = {
                "mw": pool.tile([P, 36, G], F32, name="inv_mw"),
                "cof": pool.tile([P, 9, G], F32, name="inv_cof"),
                "t2": pool.tile([P, 9, G], F32, name="inv_t2"),
                "det": pool.tile([P, G], F32, name="inv_det"),
                "idet": pool.tile([P, G], F32, name="inv_idet"),
                "p3": pool.tile([P, 3, G], F32, name="inv_p3"),
            }
            Pinv = pool.tile([P, 9, G], F32)
            Minv = pool.tile([P, 9, G], F32)
            QtPi = pool.tile([P, 9, G], F32)
            # Pool-engine congruence scratch (Htt chain runs on Pool while
            # DVE does the Hrr chain + P-inverse)
            prodP = pool.tile([P, 3, 3, 3, G], F32)
            UVP = pool.tile([P, 18, G], F32)
            rhs_t = pool.tile([P, 3, G], F32)
            dt3 = pool.tile([P, 3, G], F32)
            rhs_r = pool.tile([P, 3, G], F32)
            dr3 = pool.tile([P, 3, G], F32)

            Xg = Xt[:].rearrange("p (g n) -> p g n", g=G)
            Yg = Yt[:].rearrange("p (g n) -> p g n", g=G)
            Zg = Zt[:].rearrange("p (g n) -> p g n", g=G)
            XYZg = [Xg, Yg, Zg]

            def stt(out, in0, scalar, in1, op0, op1):
                """out = (in0 op0 scalar) op1 in1, with non-STT fallback."""
                if feat["use_stt"]:
                    nc.vector.scalar_tensor_tensor(out, in0, scalar, in1, op0, op1)
                    return
                sz = int(np.prod(in0.shape[1:]))
                tmp = sttbuf[:, 0:sz]
                if len(in0.shape) == 3:
                    tmp = tmp.rearrange("p (a b) -> p a b", a=in0.shape[1])
                nc.vector.tensor_scalar(tmp, in0, scalar, None, op0)
                nc.vector.tensor_tensor(out, tmp, in1, op1)

            def flat(t):
                return t[:].rearrange("p r g -> p (r g)")

            R9f, R9bf, nA2f = flat(R9), flat(R9b), flat(nA2)

            for rep in range(reps):
              PS = PSt[rep % 2]
              if rep + 1 < reps:
                nc.sync.dma_start(
                    out=PSt[(rep + 1) % 2][:].rearrange("p e g -> p (e g)"),
                    in_=pose_d[:])
              for it in range(niter):
                # ======== rodrigues (sin/cos on ACT) ========
                rot = PS[:, 0:3, :]
                tv = PS[:, 3:6, :]
                nc.vector.tensor_tensor(sq3[:], rot, rot, OP.mult)
                nc.vector.tensor_reduce(th2[:], sq3[:].transpose([0, 2, 1]), AX.X, OP.add)
                nc.vector.tensor_scalar(th2[:], th2[:], 1e-12, None, OP.add)
                # ith = rsqrt(th2) via bit trick + 3 Newton steps; th = th2 * ith
                nc.vector.tensor_scalar(ith[:].bitcast(I32), th2[:].bitcast(I32),
                                        1, None, OP.arith_shift_right)
                nc.vector.tensor_tensor(ith[:].bitcast(I32), c5f[:],
                                        ith[:].bitcast(I32), OP.subtract)
                for _ in range(2):
                    nc.vector.tensor_tensor(tmpg[:], ith[:], ith[:], OP.mult)
                    nc.vector.tensor_tensor(tmpg[:], tmpg[:], th2[:], OP.mult)
                    nc.vector.tensor_scalar(tmpg[:], tmpg[:], -0.5, 1.5, OP.mult, OP.add)
                    nc.vector.tensor_tensor(ith[:], ith[:], tmpg[:], OP.mult)
                nc.vector.tensor_tensor(th[:], th2[:], ith[:], OP.mult)
                if feat["use_act_sin"]:
                    # theta < ~1.7 rad here, inside the Sin table domain.
                    # omc = 1-cos = 2 sin^2(theta/2); cth = 1 - omc.
                    nc.scalar.activation(sth[:], th[:], ACTF.Sin)
                    nc.scalar.activation(s2h[:], th[:], ACTF.Sin, scale=0.5)
                    nc.vector.tensor_tensor(omc[:], s2h[:], s2h[:], OP.mult)
                    nc.vector.tensor_scalar(omc[:], omc[:], 2.0, None, OP.mult)
                    nc.vector.tensor_scalar(cth[:], omc[:], -1.0, 1.0, OP.mult, OP.add)
                else:
                    xr = sq3[:, 0, :]
                    x2 = sq3[:, 1, :]
                    nc.vector.tensor_scalar(xr, th[:], float(np.pi), None, OP.is_gt)
                    nc.vector.scalar_tensor_tensor(xr, xr, float(-2 * np.pi), th[:],
                                                   OP.mult, OP.add)
                    nc.vector.tensor_tensor(x2, xr, xr, OP.mult)
                    for dst, coef in ((sth, SIN_C), (cth, COS_C)):
                        nc.vector.tensor_scalar(dst[:], x2, coef[6], coef[5],
                                                OP.mult, OP.add)
                        for kq in (4, 3, 2, 1, 0):
                            nc.vector.tensor_tensor(dst[:], dst[:], x2, OP.mult)
                            nc.vector.tensor_scalar(dst[:], dst[:], coef[kq], None, OP.add)
                    nc.vector.tensor_tensor(sth[:], sth[:], xr, OP.mult)
                    nc.vector.tensor_scalar(omc[:], cth[:], -1.0, 1.0, OP.mult, OP.add)
                ithb = ith[:].unsqueeze(1).broadcast_to([P, 3, G])
                nc.vector.tensor_tensor(k3[:], rot, ithb, OP.mult)
                nc.vector.tensor_tensor(kkd[:], k3[:], k3[:], OP.mult)
                nc.vector.tensor_tensor(kko[:, 0:2, :], k3[:, 0:2, :], k3[:, 1:3, :], OP.mult)
                nc.vector.tensor_tensor(kko[:, 2:3, :], k3[:, 0:1, :], k3[:, 2:3, :], OP.mult)
                sb = sth[:].unsqueeze(1).broadcast_to([P, 3, G])
                nc.vector.tensor_tensor(sk[:], k3[:], sb, OP.mult)
                ob = omc[:].unsqueeze(1).broadcast_to([P, 3, G])
                nc.vector.tensor_tensor(okkd[:], kkd[:], ob, OP.mult)
                nc.vector.tensor_tensor(okko[:], kko[:], ob, OP.mult)
                cb = cth[:].unsqueeze(1).broadcast_to([P, 3, G])
                diagAP = R9[:, 0:9:4, :]
                nc.vector.tensor_tensor(diagAP, okkd[:], cb, OP.add)
                for (row, o, skr, op) in ((1, 0, 2, OP.subtract), (5, 1, 0, OP.subtract),
                                          (2, 2, 1, OP.add), (3, 0, 2, OP.add),
                                          (7, 1, 0, OP.add), (6, 2, 1, OP.subtract)):
                    nc.vector.tensor_tensor(R9[:, row:row + 1, :], okko[:, o:o + 1, :],
                                            sk[:, skr:skr + 1, :], op)

                # ======== Jr stack (J9) ========
                nc.vector.tensor_tensor(alf[:], omc[:], ith[:], OP.mult)
                nc.vector.tensor_tensor(tmpg[:], th[:], sth[:], OP.subtract)
                nc.vector.tensor_tensor(bet[:], tmpg[:], ith[:], OP.mult)
                nc.vector.tensor_scalar(omb[:], bet[:], -1.0, 1.0, OP.mult, OP.add)
                ab = alf[:].unsqueeze(1).broadcast_to([P, 3, G])
                bb = bet[:].unsqueeze(1).broadcast_to([P, 3, G])
                nc.vector.tensor_tensor(ak[:], k3[:], ab, OP.mult)
                nc.vector.tensor_tensor(bkkd[:], kkd[:], bb, OP.mult)
                nc.vector.tensor_tensor(bkko[:], kko[:], bb, OP.mult)
                obb = omb[:].unsqueeze(1).broadcast_to([P, 3, G])
                nc.vector.tensor_tensor(J9[:, 0:9:4, :], bkkd[:], obb, OP.add)
                for (row, o, akr, op) in ((1, 0, 2, OP.add), (5, 1, 0, OP.add),
                                          (2, 2, 1, OP.subtract), (3, 0, 2, OP.subtract),
                                          (7, 1, 0, OP.subtract), (6, 2, 1, OP.add)):
                    nc.vector.tensor_tensor(J9[:, row:row + 1, :], bkko[:, o:o + 1, :],
                                            ak[:, akr:akr + 1, :], op)

                # Normalized camera coords (uv0 pre-transformed to (uv-c)/f on
                # host): A = R, b = t, so no K*R lincombs. H and g both scale
                # by f^2, leaving the damped solve unchanged (validated).
                nc.vector.tensor_scalar(nA2[:], R9[:, 6:9, :], -1.0, None,
                                        OP.mult)
                PSf = PS[:].rearrange("p e g -> p (e g)")

                # ======== projection p = R x + t ========
                # ACT seeds dst = R_c2*z + t_c per (c,g) (per-partition scalar
                # APs force per-g ops); DVE then accumulates the X/Y terms for
                # all g at once with free-axis-broadcast R entries.
                if rep == 0 and it == 0:
                    emit_pretouch()
                p014 = p01[:].rearrange("p g (c n) -> p g c n", c=2)
                # z-row (c=2) first: the reciprocal then overlaps the x/y-row
                # accumulates instead of waiting for the whole projection
                for c in (2, 0, 1):
                    dst_all = p2t[:] if c == 2 else p014[:, :, c, :]
                    for g in range(G):
                        dst = p2t[:, g, :] if c == 2 else p01[:, g, c * NPT:(c + 1) * NPT]
                        s_z = R9f[:, (3 * c + 2) * G + g:(3 * c + 2) * G + g + 1]
                        s_b = PSf[:, (3 + c) * G + g:(3 + c) * G + g + 1]
                        if feat["use_act_proj"]:
                            nc.scalar.activation(dst, Zg[:, g, :], ACTF.Identity,
                                                 bias=s_b, scale=s_z)
                        else:
                            nc.vector.tensor_scalar(dst, Zg[:, g, :], s_z, s_b,
                                                    OP.mult, OP.add)
                    a0b = R9[:, 3 * c, :].unsqueeze(2).broadcast_to([P, G, NPT])
                    a1b = R9[:, 3 * c + 1, :].unsqueeze(2).broadcast_to([P, G, NPT])
                    nc.vector.tensor_tensor(paddS[c % 2][:], Xg, a0b, OP.mult)
                    nc.vector.tensor_tensor(paddS[2][:], Yg, a1b, OP.mult)
                    nc.vector.tensor_tensor(dst_all, dst_all, paddS[c % 2][:], OP.add)
                    nc.vector.tensor_tensor(dst_all, dst_all, paddS[2][:], OP.add)
                    if c == 2:
                        with nc.allow_low_precision(reason="iz consumed as bf16"):
                            nc.vector.reciprocal(
                                izt16[:].rearrange("p g n -> p (g n)"),
                                p2t[:].rearrange("p g n -> p (g n)"))

                def v4(t):
                    return t[:].rearrange("p g (s n) -> p g s n", s=2)

                izb = izt16[:].unsqueeze(2).broadcast_to([P, G, 2, NPT])
                nc.vector.tensor_tensor(v4(uvt), v4(p01), izb, OP.mult)

                # ======== E rows: e_sk = (uv_s * (-A2k) + A_sk) * iz ========
                # the last use_pool_e groups run on Pool (2-inst form) while
                # DVE handles the rest with the fused AFFINE_MUL custom op
                from concourse.dve_ops import AFFINE_MUL_REDUCE as _AMR
                n_pool_e = int(feat["use_pool_e"]) if feat["use_pool_split"] else 0
                for kk in range(3):
                    for s in range(2):
                        for g in range(G):
                            eo = E[kk][:, g, s * NPT:(s + 1) * NPT]
                            ei = uvt[:, g, s * NPT:(s + 1) * NPT]
                            s0 = nA2f[:, kk * G + g:kk * G + g + 1]
                            s1 = R9f[:, (3 * s + kk) * G + g:(3 * s + kk) * G + g + 1]
                            if g >= G - n_pool_e:
                                nc.gpsimd.tensor_scalar(ptmp[:], ei, s0, s1,
                                                        OP.mult, OP.add)
                                nc.gpsimd.tensor_tensor(eo, ptmp[:],
                                                        izt16[:, g, :], OP.mult)
                            elif feat["use_affine"]:
                                nc.vector._custom_dve(
                                    _AMR, out=eo, in0=ei,
                                    in1=izt16[:, g, :], s0=s0, s1=s1)
                            else:
                                tmp = sttbuf[:, 0:NPT].bitcast(BF16)[:, 0:NPT]
                                nc.vector.tensor_scalar(tmp, ei, s0, s1,
                                                        OP.mult, OP.add)
                                nc.vector.tensor_tensor(eo, tmp, izt16[:, g, :],
                                                        OP.mult)

                # ======== F rows: f_a = e_b x_c - e_c x_b (cyclic) ========
                # F2 runs on Pool (own scratch) while DVE does F0/F1; the
                # S pairs touching F2 come late enough to cover Pool's pace.
                for a in (2, 0, 1):
                    bq, cq = (a + 1) % 3, (a + 2) % 3
                    xc = XYZg[cq].unsqueeze(2).broadcast_to([P, G, 2, NPT])
                    xb = XYZg[bq].unsqueeze(2).broadcast_to([P, G, 2, NPT])
                    if a == 2 and feat["use_pool_split"]:
                        nc.gpsimd.tensor_tensor(v4(pfc1), v4(E[bq]), xc, OP.mult)
                        nc.gpsimd.tensor_tensor(v4(pfc2), v4(E[cq]), xb, OP.mult)
                        nc.gpsimd.tensor_tensor(Ft[a][:], pfc1[:], pfc2[:],
                                                OP.subtract)
                    else:
                        nc.vector.tensor_tensor(v4(fcr1), v4(E[bq]), xc, OP.mult)
                        nc.vector.tensor_tensor(v4(fcr2), v4(E[cq]), xb, OP.mult)
                        nc.vector.tensor_tensor(Ft[a][:], fcr1[:], fcr2[:],
                                                OP.subtract)
                # rres feeds only the sv products at the tail of the S stage;
                # emitted after F2 so Pool's in-order queue doesn't stall the
                # DVE F rows behind it
                reng = nc.gpsimd if feat["use_pool_split"] else nc.vector
                reng.tensor_tensor(rres[:], uvt[:], UV0[:], OP.subtract)

                # ======== S = sum J^T J, s = sum J^T r ========
                # diagonals: ACT Square+accumulate straight from the J tiles;
                # n_polar off-diagonal pairs via polarization
                #   S_ab = 0.5*[(Ja+Jb)^2 - Ja^2 - Jb^2]  (DVE add, ACT square)
                # remaining pairs: DVE bf16 mult + preadd + reduce.
                Jt = [Ft[0], Ft[1], Ft[2], E[0], E[1], E[2]]
                SQ5 = float(np.sqrt(0.5))
                n_polar = int(feat["n_polar"]) if feat["use_act_diag"] else 0
                pi = 0

                n_pool_pre = int(feat["use_pool_preadd"])
                # pipeline items [dst, pr, on_pool, pd, pd2, age]: src(k)
                # [DVE mult or bulk ACT square] -> preadd(k) at k+1 [every
                # n-th on Pool] -> half-preadd(k) at k+2 -> reduce(k) at k+3.
                # Deferral absorbs both Pool's slower preadds and ACT's
                # square latency; the 1x-rate tensor_reduce sees NPT/2 elems.
                mr_pending = []  # [dst, pr, on_pool, pd, pd2, pd3, age]
                mr_ctr = [0, 0, 0, 0]  # [pool-pd, dve-pd, pd2, pd3] counters
                H2, H4 = NPT // 2, NPT // 4

                def _mr_tick():
                    for item in mr_pending:
                        item[6] += 1
                    for item in list(mr_pending):
                        if item[5] is not None and item[6] >= 4:
                            nc.vector.tensor_reduce(item[0], item[5][:],
                                                    AX.X, OP.add)
                            mr_pending.remove(item)
                            break
                    for item in mr_pending:
                        if item[4] is not None and item[5] is None and item[6] >= 3:
                            pd3 = padd3[mr_ctr[3] % 3]
                            mr_ctr[3] += 1
                            nc.vector.tensor_tensor(pd3[:],
                                                    item[4][:, :, 0:H4],
                                                    item[4][:, :, H4:H2],
                                                    OP.add)
                            item[5] = pd3
                            break
                    for item in mr_pending:
                        if item[3] is not None and item[4] is None and item[6] >= 2:
                            pd2 = padd2[mr_ctr[2] % 3]
                            mr_ctr[2] += 1
                            nc.vector.tensor_tensor(pd2[:],
                                                    item[3][:, :, 0:H2],
                                                    item[3][:, :, H2:NPT],
                                                    OP.add)
                            item[4] = pd2
                            break
                    for item in mr_pending:
                        if item[3] is None and item[6] >= 1:
                            pr, on_pool = item[1], item[2]
                            if on_pool:
                                pd = ppadd[mr_ctr[0] % 3]
                                mr_ctr[0] += 1
                            else:
                                pd = paddS[mr_ctr[1] % 3]
                                mr_ctr[1] += 1
                            eng = nc.gpsimd if on_pool else nc.vector
                            eng.tensor_tensor(pd[:], pr[:, :, 0:NPT],
                                              pr[:, :, NPT:2 * NPT], OP.add)
                            item[3] = pd
                            break

                def mr_flush():
                    while mr_pending:
                        _mr_tick()

                def _mr_push(dst, pr, on_pool):
                    _mr_tick()
                    mr_pending.append([dst, pr, on_pool, None, None, None, 0])

                def mult_reduce(dst, in_a, in_b):
                    nonlocal pi
                    on_pool = (n_pool_pre > 0 and pi % 3 == 0
                               and pi // 3 < n_pool_pre)
                    pr = prodS[pi % 3]
                    pi += 1
                    nc.vector.tensor_tensor(pr[:], in_a[:], in_b[:], OP.mult)
                    _mr_push(dst, pr, on_pool)

                sqb = [pool.tile([P, G, 2 * NPT], BF16, name=f"sqb{i}")
                       for i in range(3)]
                sq_ctr = [0]

                def square_reduce(dst, in_a):
                    # diag entry: ONE bulk elementwise Square on ACT (no per-g
                    # accum_out — read-accumulator overhead makes that ~3x
                    # slower), then the shared preadd/reduce pipeline.
                    pr = sqb[sq_ctr[0] % 3]
                    sq_ctr[0] += 1
                    nc.scalar.activation(
                        pr[:].rearrange("p g n -> p (g n)"),
                        in_a[:].rearrange("p g n -> p (g n)"), ACTF.Square)
                    _mr_push(dst, pr, False)

                # index 2 (Ft[2]) is Pool-computed and lands last; order all
                # in-order engine streams so their Ft[2] consumers come last
                diag_order = [0, 1, 3, 4, 5, 2]
                offdiag = [(0, 1), (0, 3), (0, 4), (0, 5), (1, 3), (1, 4),
                           (1, 5), (3, 4), (3, 5), (4, 5),
                           (0, 2), (1, 2), (2, 3), (2, 4), (2, 5)]
                polar = []
                n_diag_bulk = int(feat.get("n_diag_bulk", 0))
                if feat["use_act_diag"]:
                    for di, a in enumerate(diag_order):
                        if di < n_diag_bulk:
                            square_reduce(SST[:, 7 * a, :], Jt[a])
                        else:
                            for g in range(G):
                                nc.scalar.activation(
                                    act_sink[:], Jt[a][:, g, :], ACTF.Square,
                                    accum_out=SST[:, 7 * a, g:g + 1])
                    n_pool_polar = int(feat.get("n_pool_polar", 0))
                    for (a, bq) in offdiag[:n_polar]:
                        ps = psum16[len(polar) % len(psum16)]
                        # last k pairs' sums on Pool: their ACT squares come
                        # late in ACT's queue, covering Pool's slower add
                        peng = (nc.gpsimd
                                if len(polar) >= n_polar - n_pool_polar
                                else nc.vector)
                        peng.tensor_tensor(ps[:], Jt[a][:], Jt[bq][:],
                                           OP.add)
                        qrow = len(polar)
                        for g in range(G):
                            nc.scalar.activation(
                                act_sink[:], ps[:, g, :], ACTF.Square,
                                scale=SQ5,
                                accum_out=Qpol[:, qrow, g:g + 1])
                        polar.append((a, bq, qrow))
                else:
                    for a in range(6):
                        mult_reduce(SST[:, 7 * a, :], Jt[a], Jt[a])
                for (a, bq) in offdiag[n_polar:]:
                    mult_reduce(SST[:, 6 * a + bq, :], Jt[a], Jt[bq])
                # sv: first n_sv_act entries go product-on-DVE + Copy-accum
                # reduce on ACT (balancing the engines); rest are mult_reduce
                n_sv_act = int(feat.get("n_sv_act", 0))
                for ai, a in enumerate(diag_order[:n_sv_act]):
                    pr = svpr[ai % 3]
                    nc.vector.tensor_tensor(pr[:], Jt[a][:], rres[:], OP.mult)
                    for g in range(G):
                        nc.scalar.activation(act_sink[:], pr[:, g, :],
                                             ACTF.Copy,
                                             accum_out=sv[:, a, g:g + 1])
                for a in diag_order[n_sv_act:]:
                    mult_reduce(sv[:, a, :], Jt[a], rres)
                mr_flush()
                # combine polarized pairs: S_ab = Q' - 0.5*Da - 0.5*Db
                if polar:
                    Dh = inv_ws["cof"]  # scratch [P, 9, G], unused until _inv3
                    for a in range(6):
                        nc.vector.tensor_scalar(Dh[:, a, :], SST[:, 7 * a, :],
                                                0.5, None, OP.mult)
                    for (a, bq, qrow) in polar:
                        nc.vector.tensor_tensor(tmpg[:], Dh[:, a, :], Dh[:, bq, :],
                                                OP.add)
                        nc.vector.tensor_tensor(SST[:, 6 * a + bq, :],
                                                Qpol[:, qrow, :], tmpg[:],
                                                OP.subtract)
                # mirror lower triangle: rows 7a+d -> 7a+6d, a<6-d
                # (on DVE: by this point DVE has already waited on the ACT and
                # Pool clocks, so these carry no extra sem waits — the walrus
                # build allows at most one wait per instruction)
                for d in range(1, 6):
                    n = 6 - d
                    nc.vector.tensor_copy(SST[:, 6 * d:6 * d + 7 * (n - 1) + 1:7, :],
                                          SST[:, d:d + 7 * (n - 1) + 1:7, :])

                # ======== congruence H = W^T S W (W = blockdiag(Jr, R^T)) ========
                SS4 = SST[:].rearrange("p (a l) g -> p a l g", a=6)
                Srr = SS4[:, 0:3, 0:3, :]
                Srt = SS4[:, 0:3, 3:6, :]
                Stt = SS4[:, 3:6, 3:6, :]
                # Htt chain (UV block, Q9, M9) on Pool; Hrr chain + inverses
                # on DVE; gr3/gt3 fill DVE's wait on Pool's Q9/M9.
                A6 = SS4[:, 0:6, 3:6, :]                    # [p, 6, 3, g]
                # UV = [Srt; Stt] @ R^T: B[l,b] = R[b,l] via transposed view
                RT4 = _stack3(R9).transpose([0, 2, 1, 3])
                prodP6 = prodP[:].rearrange(
                    "p a b l g -> p (a b) l g")[:, 0:6, :, :]   # [P, 6, 3, G]
                _matmul3_nored(nc.gpsimd, prodP6, UVP[:], A6, RT4, nrow=6)
                U9v = UVP[:, 0:9, :]
                V9v = UVP[:, 9:18, :]
                prodP3 = prodP[:, 0, :, :, :]
                _matmul3_nored(nc.gpsimd, prodP3, Q9[:], _stack3(J9),
                               U9v.rearrange("p (l b) g -> p l b g", l=3),
                               transA=True)
                _matmul3_nored(nc.gpsimd, prodP3, M9[:], _stack3(R9),
                               V9v.rearrange("p (l b) g -> p l b g", l=3))
                nc.gpsimd.tensor_scalar(M9[:, 0:9:4, :], M9[:, 0:9:4, :],
                                        float(damp), None, OP.add)
                _matmul3(nc, prod, T1, Srr, J9[:])
                _matmul3(nc, prod, P9, _stack3(J9), T1[:], transA=True)  # Hrr

                # ======== damped Schur solve ========
                nc.vector.tensor_scalar(P9[:, 0:9:4, :], P9[:, 0:9:4, :],
                                        float(damp), None, OP.add)
                _inv3(nc, inv_ws, P9, Pinv, G)
                _matvec3(nc, prod, gr3[:], _stack3(J9), sv[:, 0:3, :], transA=True)
                _matvec3(nc, prod, gt3[:], _stack3(R9), sv[:, 3:6, :])
                _matmul3(nc, prod, QtPi, _stack3(Q9), Pinv[:], transA=True)
                _matmul3(nc, prod, U9, _stack3(QtPi), Q9[:], sub_from=None)
                nc.vector.tensor_tensor(M9[:], M9[:], U9[:], OP.subtract)
                _inv3(nc, inv_ws, M9, Minv, G)
                _matvec3(nc, prod, rhs_t[:], _stack3(QtPi), gr3[:], sub_from=gt3[:])
                _matvec3(nc, prod, dt3[:], _stack3(Minv), rhs_t[:])
                _matvec3(nc, prod, rhs_r[:], _stack3(Q9), dt3[:], sub_from=gr3[:])
                _matvec3(nc, prod, dr3[:], _stack3(Pinv), rhs_r[:])

                # pose update: rot += dr' (sign-flipped), t -= dt
                nc.vector.tensor_tensor(PS[:, 0:3, :], PS[:, 0:3, :], dr3[:], OP.add)
                nc.vector.tensor_tensor(PS[:, 3:6, :], PS[:, 3:6, :], dt3[:], OP.subtract)

                if rep == 0 and it == 0 and dbg_requests:
                    local = dict(R9=R9, J9=J9, nA2=nA2, p01=p01,
                                 p2t=p2t, izt=izt, uvt=uvt, rres=rres, SST=SST,
                                 sv=sv, Hrr=P9, Q9=Q9, Htt=M9, gr3=gr3, gt3=gt3,
                                 Pinv=Pinv, Minv=Minv, QtPi=QtPi, dt3=dt3, dr3=dr3,
                                 th=th, sth=sth, cth=cth, k3=k3,
                                 E0=E[0], E1=E[1], E2=E[2],
                                 F0=Ft[0], F1=Ft[1], F2=Ft[2])
                    for nm in dbg_requests:
                        t = local[nm]
                        ap = t[:]
                        fshape = [P, ap.free_size()]
                        dram = nc.declare_dram_parameter(f"dbg_{nm}", fshape, F32,
                                                         isOutput=True)
                        flatap = ap
                        while len(flatap.shape) > 2:
                            flatap = flatap.rearrange(
                                "p " + " ".join(f"d{i}" for i in range(len(flatap.shape) - 1))
                                + " -> p (" + " ".join(f"d{i}" for i in range(len(flatap.shape) - 1)) + ")")
                        if t[:].dtype != F32:
                            cv = pool.tile(fshape, F32, name=f"dbgc_{nm}")
                            nc.vector.tensor_copy(cv[:], flatap)
                            flatap = cv[:]
                        nc.sync.dma_start(out=dram[:], in_=flatap)
                        dbg_tiles[nm] = fshape

              for fz in range(frozen):
                # ==== pass B: reproject at current pose; J, W, H^-1 frozen ====
                # Fixed point solves J(p0)^T r(p*) = 0; H only preconditions,
                # so the E/F tiles, congruence W(p0) and Schur pieces are all
                # reused from the full iteration (study: rel 2.1e-3 vs 8-iter
                # reference for 1 full + 1 frozen, 10x inside the 2e-2 gate).
                #
                # Rotation via the right-Jacobian identity instead of a full
                # rodrigues recompute: R(r0+dr) = R(r0) exp([Jr(r0) dr]x),
                # 2nd-order exp; error O(|dr|^3) ~ 2e-5 (validated: 2.075e-3
                # vs 2.098e-3 with exact rodrigues).
                w3 = k3   # scratch [P,3,G], free in this pass
                M9b = U9  # scratch [P,9,G], free in this pass
                _matvec3(nc, prod, w3[:], _stack3(J9), dr3[:])
                Mb4 = _stack3(M9b)
                wl = w3[:].unsqueeze(2).broadcast_to([P, 3, 3, G])
                wc = w3[:].unsqueeze(1).broadcast_to([P, 3, 3, G])
                nc.vector.tensor_tensor(Mb4, wl, wc, OP.mult)      # w_l w_b
                nc.vector.tensor_scalar(M9b[:], M9b[:], 0.5, None, OP.mult)
                Mdiag = M9b[:, 0:9:4, :]
                nc.vector.tensor_reduce(th2[:], Mdiag.transpose([0, 2, 1]),
                                        AX.X, OP.add)              # 0.5|w|^2
                nc.vector.tensor_scalar(tmpg[:], th2[:], -1.0, 1.0,
                                        OP.mult, OP.add)           # 1 - 0.5|w|^2
                tb = tmpg[:].unsqueeze(1).broadcast_to([P, 3, G])
                nc.vector.tensor_tensor(Mdiag, Mdiag, tb, OP.add)
                for (row, wi, op) in ((1, 2, OP.subtract), (2, 1, OP.add),
                                      (3, 2, OP.add), (5, 0, OP.subtract),
                                      (6, 1, OP.subtract), (7, 0, OP.add)):
                    nc.vector.tensor_tensor(M9b[:, row:row + 1, :],
                                            M9b[:, row:row + 1, :],
                                            w3[:, wi:wi + 1, :], op)
                _matmul3(nc, prod, R9b, _stack3(R9), M9b[:])
                # projection at current pose (normalized coords: A=R(p), b=t);
                # ACT (idle in this pass) does the z-row affine seed, DVE the
                # all-g broadcast X/Y accumulates
                PSfb = PS[:].rearrange("p e g -> p (e g)")
                p014b = p01[:].rearrange("p g (c n) -> p g c n", c=2)
                # z-row seeds on DVE (cheap 4x TSP) so its accumulates and
                # the reciprocal never wait on ACT's seed chain; x/y rows
                # seed on ACT in parallel
                for c in (2, 0, 1):
                    dst_all = p2t[:] if c == 2 else p014b[:, :, c, :]
                    for g in range(G):
                        dst = p2t[:, g, :] if c == 2 else p01[:, g, c * NPT:(c + 1) * NPT]
                        s_z = R9bf[:, (3 * c + 2) * G + g:(3 * c + 2) * G + g + 1]
                        s_b = PSfb[:, (3 + c) * G + g:(3 + c) * G + g + 1]
                        if c == 2:
                            nc.vector.tensor_scalar(dst, Zg[:, g, :], s_z, s_b,
                                                    OP.mult, OP.add)
                        else:
                            nc.scalar.activation(dst, Zg[:, g, :], ACTF.Identity,
                                                 bias=s_b, scale=s_z)
                    a0b = R9b[:, 3 * c, :].unsqueeze(2).broadcast_to([P, G, NPT])
                    a1b = R9b[:, 3 * c + 1, :].unsqueeze(2).broadcast_to([P, G, NPT])
                    nc.vector.tensor_tensor(paddS[c % 2][:], Xg, a0b, OP.mult)
                    nc.vector.tensor_tensor(paddS[2][:], Yg, a1b, OP.mult)
                    nc.vector.tensor_tensor(dst_all, dst_all, paddS[c % 2][:], OP.add)
                    nc.vector.tensor_tensor(dst_all, dst_all, paddS[2][:], OP.add)
                    if c == 2:
                        with nc.allow_low_precision(reason="iz consumed as bf16"):
                            nc.vector.reciprocal(
                                izt16[:].rearrange("p g n -> p (g n)"),
                                p2t[:].rearrange("p g n -> p (g n)"))
                izb2 = izt16[:].unsqueeze(2).broadcast_to([P, G, 2, NPT])
                nc.vector.tensor_tensor(v4(uvt), v4(p01), izb2, OP.mult)
                # rres on DVE here: it gates every sv product, and Pool would
                # take ~4x longer
                nc.vector.tensor_tensor(rres[:], uvt[:], UV0[:], OP.subtract)
                # sv_a = sum_pts J_a(p0) * r — first n_sv_act_b entries reduce
                # on ACT (idle after the projection seeds); rest on DVE with
                # alternating Pool/DVE preadds, reduce deferred one step
                nsab = int(feat.get("n_sv_act_b", 0))
                sv_order = (0, 1, 3, 4, 5, 2)
                for ai, a in enumerate(sv_order[:nsab]):
                    pr = svpr[ai % 3]
                    nc.vector.tensor_tensor(pr[:], Jt[a][:], rres[:], OP.mult)
                    for g in range(G):
                        nc.scalar.activation(act_sink[:], pr[:, g, :],
                                             ACTF.Copy,
                                             accum_out=sv[:, a, g:g + 1])
                for a in sv_order[nsab:]:
                    mult_reduce(sv[:, a, :], Jt[a], rres)
                mr_flush()
                # g = W^T s and the frozen Schur solve
                _matvec3(nc, prod, gr3[:], _stack3(J9), sv[:, 0:3, :], transA=True)
                _matvec3(nc, prod, gt3[:], _stack3(R9), sv[:, 3:6, :])
                _matvec3(nc, prod, rhs_t[:], _stack3(QtPi), gr3[:], sub_from=gt3[:])
                _matvec3(nc, prod, dt3[:], _stack3(Minv), rhs_t[:])
                _matvec3(nc, prod, rhs_r[:], _stack3(Q9), dt3[:], sub_from=gr3[:])
                _matvec3(nc, prod, dr3[:], _stack3(Pinv), rhs_r[:])
                nc.vector.tensor_tensor(PS[:, 0:3, :], PS[:, 0:3, :], dr3[:], OP.add)
                nc.vector.tensor_tensor(PS[:, 3:6, :], PS[:, 3:6, :], dt3[:], OP.subtract)

            nc.sync.dma_start(out=out_d[:], in_=PS[:].rearrange("p e g -> p (e g)"))
            # DMA-completion observability chain (see _patch_tail_drain)
            jrd = pool.tile([P, 6], F32)
            jrd2 = pool.tile([P, 6], F32)
            nc.sync.dma_start(out=jrd[:], in_=out_d[:, 0:6])
            nc.vector.tensor_copy(jrd2[:], jrd[:])

    from concourse.library_overlay import lower_extended_insts
    lower_extended_insts(nc)
    # Split multi-sem waits into InstEventSemaphore pairs: this walrus build
    # rejects >1 raw sem wait per instruction, and the cross-engine pipeline
    # (DVE/Pool/ACT) legitimately produces a few two-wait joins.
    import bass_rust as _bass_rust
    _bass_rust.generate_event_semaphores(nc)
    return nc


# ---------------------------------------------------------------------------
# host-side sharding + execution
# ---------------------------------------------------------------------------

def _normalize_uv(pts2d, K):
    """Pixel -> normalized camera coords: (uv - c) / f. The device kernel
    works K-free (A=R, b=t); H and g scale by f^2, so the damped GN step is
    unchanged."""
    K = np.asarray(K, np.float64)
    f = np.array([K[0, 0], K[1, 1]])
    c = np.array([K[0, 2], K[1, 2]])
    return ((np.asarray(pts2d, np.float64) - c) / f).astype(np.float32)


def _shard_core(pts2d_c, pts3d_c, init_pose_c, G):
    xyz = pts3d_c.reshape(G, P, NPT, 3).transpose(3, 1, 0, 2).reshape(3, P, G * NPT)
    uv0 = pts2d_c.reshape(G, P, NPT, 2).transpose(1, 0, 3, 2).reshape(P, G * 2 * NPT)
    pose0 = init_pose_c.reshape(G, P, 6).transpose(1, 2, 0).reshape(P, 6 * G)
    inp16 = np.concatenate([xyz[0], xyz[1], xyz[2], uv0], axis=1)
    return {"inp": np.ascontiguousarray(inp16).astype(ml_dtypes.bfloat16),
            "pose0": np.ascontiguousarray(pose0, np.float32)}


def _unshard_core(pose_out, G):
    return pose_out.reshape(P, 6, G).transpose(2, 0, 1).reshape(G * P, 6)


def kernel(pts2d, pts3d, K, init_pose):
    pts2d = np.asarray(pts2d, np.float32)
    pts3d = np.asarray(pts3d, np.float32)
    K = np.asarray(K, np.float32)
    init_pose = np.asarray(init_pose, np.float32)

    batch = pts3d.shape[0]
    bpc = batch // NCORES
    G = bpc // P

    nc = build_nc(K, G=G)
    pts2d_n = _normalize_uv(pts2d, K)
    in_maps = [
        _shard_core(pts2d_n[c * bpc:(c + 1) * bpc], pts3d[c * bpc:(c + 1) * bpc],
                    init_pose[c * bpc:(c + 1) * bpc], G)
        for c in range(NCORES)
    ]
    res = run_bass_kernel_spmd(nc, in_maps, list(range(NCORES)))
    outs = [_unshard_core(res.results[c]["pose_out"], G) for c in range(NCORES)]
    return np.concatenate(outs, axis=0).astype(np.float32)


if __name__ == "__main__":
    rng = np.random.default_rng(0)
    Km = np.array([[800.0, 0, 320.0], [0, 800.0, 240.0], [0, 0, 1.0]], np.float32)
    pts3d = rng.standard_normal((8192, 128, 3)).astype(np.float32)
    pose = np.concatenate([0.2 * rng.standard_normal((8192, 3)),
                           0.3 * rng.standard_normal((8192, 2)),
                           6 + 0.5 * rng.random((8192, 1))], axis=1).astype(np.float32)
    pts2d = rng.standard_normal((8192, 128, 2)).astype(np.float32) * 100
    out = kernel(pts2d, pts3d, Km, pose)
    print(out.shape, out.dtype, np.isfinite(out).mean())

